# revision 45
# baseline (speedup 1.0000x reference)
"""Tensor-parallel FlashLlamaAttention kernel for 8 Trainium2 NeuronCores.

Sharding: each core owns 4 query heads (512 proj dims) and 1 kv head
(128 dims). Per-core device program computes qkv projection (+RoPE),
causal GQA attention and its o_proj partial product; the 8 partial
[2048, 4096] outputs are summed on the host (replaces the all-reduce).

Device-side layouts are all "feature on partitions" (transposed), so the
host wrapper pre-transposes hidden_states and the weight shards.
All matmul operands are bf16 (f32 PSUM accumulation): bf16 enables the
PE fast-weight-load path, so LDWEIGHTS hides behind the matmul stream
instead of serializing with it (f32 weights load at 1 elem/cycle and
cost ~180ns per 128x128 tile). Rotate-half for RoPE runs on the PE as a
permutation matmul (DVE cannot move data across partitions).

Softmax denominators are batched per (batch, q-tile) into a [4, 512]
tile and inverted with one reciprocal_approx_fast (the plain DVE
reciprocal costs ~3.3us per call); attention PSUM is evacuated with
plain copies so the normalization chain stays off the PE critical path.
"""
import sys

sys.path.insert(0, "/opt/trn_rl_repo")

from contextlib import ExitStack

import numpy as np
import ml_dtypes

import concourse.bass as bass
import concourse.bacc as bacc
import concourse.mybir as mybir
import concourse.tile as tile
from concourse.bass_utils import run_bass_kernel_spmd
from concourse.masks import make_identity

F32 = mybir.dt.float32
BF = mybir.dt.bfloat16
EXP = mybir.ActivationFunctionType.Exp

P = 128          # partitions / head dim
T = 2048         # total tokens (B * S)
S = 1024         # seq len per batch
B = 2
HD = 4096        # hidden dim
NHL = 4          # local query heads per core
DQKV = NHL * P + P + P  # 768 local projection dims (4q + k + v)
SM = float(P) ** -0.5

N_CORES = 8


def build_nc():
    nc = bacc.Bacc("TRN2", target_bir_lowering=False, debug=False,
                   num_devices=N_CORES)

    hiddenT = nc.dram_tensor("hiddenT", [HD, T], BF, kind="ExternalInput").ap()
    wqkvT = nc.dram_tensor("wqkvT", [HD, DQKV], BF, kind="ExternalInput").ap()
    woT = nc.dram_tensor("woT", [NHL * P, HD], BF, kind="ExternalInput").ap()
    cosF = nc.dram_tensor("cosF", [P, T], F32, kind="ExternalInput").ap()
    sinF = nc.dram_tensor("sinF", [P, T], F32, kind="ExternalInput").ap()
    out = nc.dram_tensor("out", [T, HD], BF, kind="ExternalOutput").ap()

    with tile.TileContext(nc) as tc, ExitStack() as stack:
        const = stack.enter_context(tc.tile_pool(name="const", bufs=1))
        ident_f = const.tile([P, P], F32)
        make_identity(nc, ident_f[:])
        ident = const.tile([P, P], BF)
        nc.vector.tensor_copy(ident[:], ident_f[:])
        # rotate-half permutation: perm[k, i] = 1 iff |k - i| == 64
        perm = const.tile([P, P], BF)
        nc.vector.tensor_copy(perm[:, 0:64], ident[:, 64:128])
        nc.vector.tensor_copy(perm[:, 64:128], ident[:, 0:64])
        ones_f32 = const.tile([P, 1], F32)
        nc.vector.memset(ones_f32[:], 1.0)
        ones_k = const.tile([P, 1], BF)
        nc.vector.tensor_copy(ones_k[:], ones_f32[:])
        # causal additive mask weight: triW[p, m] = -1e9 iff m > p.
        # Accumulated into the diagonal score tiles on the PE via
        # matmul(st, triW, ident) so the DVE never touches pexp.
        tri_f = const.tile([P, P], F32)
        nc.gpsimd.memset(tri_f[:], -1e9)
        nc.gpsimd.affine_select(
            out=tri_f[:], in_=tri_f[:], compare_op=mybir.AluOpType.is_ge,
            fill=0.0, base=-1, pattern=[[1, P]], channel_multiplier=-1)
        triW = const.tile([P, P], BF)
        nc.vector.tensor_copy(triW[:], tri_f[:])

        # long-lived activations, split per batch so attention on batch 0
        # never waits (tile-granular deps) on batch 1's rope/transpose work
        qkv_pool = stack.enter_context(tc.tile_pool(name="qkv", bufs=1))
        qT = [[qkv_pool.tile([P, S], BF, tag=f"qT{h}_{b}", name=f"qT{h}_{b}")
               for b in range(B)] for h in range(NHL)]
        kT = [qkv_pool.tile([P, S], BF, tag=f"kT{b}", name=f"kT{b}")
              for b in range(B)]
        v_pool = stack.enter_context(tc.tile_pool(name="v", bufs=1))
        v_sb = [v_pool.tile([P, S // P, P], BF, tag=f"v_sb{b}",
                            name=f"v_sb{b}") for b in range(B)]

        # o_proj weights tile; DMA issued inside phase 1 after the
        # wqkv/cos/sin loads so it drains during phase 1 without
        # delaying the first projection matmuls
        w2_pool = stack.enter_context(tc.tile_pool(name="w2", bufs=1))
        wo_sb = w2_pool.tile([P, NHL, HD], BF)
        woT_r = woT.rearrange("(a p) o -> p a o", p=P)

        # ---------------- phase 1: qkv projection + rope -----------------
        with (
            tc.tile_pool(name="cs", bufs=1) as cs_pool,
            tc.tile_pool(name="w1", bufs=1) as w1_pool,
            tc.tile_pool(name="xt", bufs=12) as xt_pool,
            tc.tile_pool(name="rot", bufs=2) as rot_pool,
            tc.tile_pool(name="qraw", bufs=5) as qraw_pool,
            tc.tile_pool(name="vtmp", bufs=1) as vtmp_pool,
            tc.tile_pool(name="pps", bufs=1, space="PSUM") as proj_psum,
            tc.tile_pool(name="rps", bufs=2, space="PSUM") as rope_psum,
        ):
            # one tile per ho so the first matmuls only depend on a 196KB
            # DMA, not the whole 6 MB weight load (deps are tile-granular)
            wqkvT_r = wqkvT.rearrange("(a p) j -> p a j", p=P)
            wqkv_g = []
            for g in range(HD // P):
                wg = w1_pool.tile([P, DQKV], BF, tag=f"wq{g}",
                                  name=f"wq{g}")
                nc.scalar.dma_start(wg[:], wqkvT_r[:, g, :])
                wqkv_g.append(wg)
            cos_sb = cs_pool.tile([P, T], F32)
            sin_sb = cs_pool.tile([P, T], F32)
            nc.scalar.dma_start(cos_sb[:], cosF[:])
            nc.scalar.dma_start(sin_sb[:], sinF[:])
            vT_tmp = vtmp_pool.tile([P, T], BF, tag="vT", name="vT_tmp")

            for tc4 in range(T // 512):
                ts = slice(512 * tc4, 512 * (tc4 + 1))
                cb = tc4 // 2          # batch this chunk belongs to
                lts = slice(512 * (tc4 % 2), 512 * (tc4 % 2 + 1))
                dsts = [(h, qT[h][cb]) for h in range(NHL)] + [(4, kT[cb])]
                ps = [proj_psum.tile([P, 512], F32, tag=f"pps{ot}",
                                     name=f"pps{ot}_{tc4}")
                      for ot in range(6)]
                for ho in range(HD // P):
                    xt = xt_pool.tile([P, 512], BF, tag="xt")
                    nc.sync.dma_start(xt[:], hiddenT[P * ho:P * (ho + 1), ts])
                    for ot in range(6):
                        nc.tensor.matmul(
                            ps[ot][:],
                            wqkv_g[ho][:, P * ot:P * (ot + 1)],
                            xt[:], start=(ho == 0), stop=(ho == HD // P - 1))
                # evacuate all 6 PSUM banks first (qraw copy + cos-mult
                # are each bank's only readers), so the next chunk's
                # matmuls unblock ASAP; the rot/sin/add RoPE work runs
                # afterwards off-PSUM
                nc.vector.tensor_copy(vT_tmp[:, ts], ps[5][:])
                qraws = []
                for idx, dst in dsts:
                    qraw = qraw_pool.tile([P, 512], BF, tag="qraw",
                                          name=f"qraw{idx}_{tc4}")
                    nc.scalar.copy(qraw[:], ps[idx][:])
                    qraws.append(qraw)
                    nc.vector.tensor_mul(out=dst[:, lts], in0=ps[idx][:],
                                         in1=cos_sb[:, ts])
                for qraw, (idx, dst) in zip(qraws, dsts):
                    rot_ps = rope_psum.tile([P, 512], F32, tag="rotp")
                    nc.tensor.matmul(rot_ps[:], perm[:], qraw[:],
                                     start=True, stop=True)
                    rt = rot_pool.tile([P, 512], F32, tag="rot")
                    nc.vector.tensor_mul(out=rt[:], in0=rot_ps[:],
                                         in1=sin_sb[:, ts])
                    nc.vector.tensor_add(out=dst[:, lts], in0=dst[:, lts],
                                         in1=rt[:])
                # transpose this chunk of V: vT [j, t] -> v_sb [t, tchunk, j]
                # via DMA xbar transpose -- keeps it off the PE/DVE/ACT
                # engines and off the rope PSUM banks entirely
                for tt in range(4 * tc4, 4 * (tc4 + 1)):
                    nc.scalar.dma_start(v_sb[cb][:, tt - 8 * cb, :],
                                        vT_tmp[:, P * tt:P * (tt + 1)],
                                        transpose=True)

        # ----- long-lived attention output (allocated after phase 1 frees)
        at_pool = stack.enter_context(tc.tile_pool(name="at", bufs=1))
        ATn = [at_pool.tile([P, T], BF, tag=f"ATn{h}", name=f"ATn{h}")
               for h in range(NHL)]
        outb_pool = stack.enter_context(tc.tile_pool(name="ob", bufs=2))

        # o_proj weights load at phase-2 start on the sync queue (idle by
        # now): the 8MB of HBM traffic lands during attention, well
        # before the first oproj group needs it
        for g in range(NHL):
            nc.sync.dma_start(wo_sb[:, g, :], woT_r[:, g, :])

        # ---------------- phase 2: causal GQA attention --------------
        if True:
            with (
                tc.tile_pool(name="pexp", bufs=6) as pexp_pool,
                tc.tile_pool(name="araw", bufs=8) as araw_pool,
                tc.tile_pool(name="dens", bufs=8) as den_small,
                tc.tile_pool(name="rden", bufs=3) as rden_pool,
                tc.tile_pool(name="stp", bufs=2, space="PSUM") as st_psum,
                tc.tile_pool(name="atp", bufs=2, space="PSUM") as at_psum,
                tc.tile_pool(name="dnp", bufs=2, space="PSUM") as den_psum,
                tc.tile_pool(name="opp", bufs=2, space="PSUM") as op_psum,
            ):
                # per (b, qt) group state: at_raw tiles + den reciprocals
                grp_state = {}

                def evac_group(b, qt, h, at_ps, den_ps):
                    """copy one finished (b, qt, h) accumulation to SBUF"""
                    st_ = grp_state[(b, qt)]
                    araw = araw_pool.tile([P, 512], F32, tag="araw",
                                          name=f"araw{b}_{qt}_{h}")
                    nc.scalar.copy(araw[:], at_ps[:])
                    den_c = den_small.tile([1, 512], F32, tag="den",
                                           name=f"den{b}_{qt}_{h}")
                    nc.scalar.copy(den_c[:], den_ps[:])
                    rec = den_small.tile([1, 512], F32, tag="rec",
                                         name=f"rec{b}_{qt}_{h}")
                    nc.vector.reciprocal_approx_fast(rec[:], den_c[:])
                    st_["araw"][h] = araw
                    st_["rec"][h] = rec

                def flush(item):
                    b, qt, h, a0, qo0, nk, px0, at_ps, den_ps = item
                    nc.tensor.matmul(
                        at_ps[:, qo0:], v_sb[b][:, a0, :],
                        px0[:, qo0:], start=(a0 == 0), stop=(a0 == nk - 1))
                    nc.tensor.matmul(
                        den_ps[:, qo0:], ones_k[:],
                        px0[:, qo0:], start=(a0 == 0), stop=(a0 == nk - 1))
                    if a0 == nk - 1:
                        evac_group(b, qt, h, at_ps, den_ps)

                pending = []       # score tiles awaiting their AV/den matmul

                def attn_group(b, qt):
                    """issue the 4 heads' score/exp/AV/den for one q tile"""
                    grp_state[(b, qt)] = {"araw": {}, "rec": {}}
                    nk = 4 * qt + 4
                    for h in range(NHL):
                        qTb = qT[h][b][:]
                        kTb = kT[b][:]
                        at_ps = at_psum.tile([P, 512], F32, tag="at")
                        den_ps = den_psum.tile([1, 512], F32, tag="den")
                        for a in range(nk):
                            qoff = max(0, P * a - 512 * qt)
                            diag = P * a >= 512 * qt
                            st = st_psum.tile([P, 512], F32, tag="st")
                            nc.tensor.matmul(
                                st[:, qoff:],
                                kTb[:, P * a:P * (a + 1)],
                                qTb[:, 512 * qt + qoff:512 * (qt + 1)],
                                start=True, stop=not diag)
                            if diag:
                                # st[k, q] += -1e9 for k > q in the
                                # diagonal 128x128 block
                                nc.tensor.matmul(
                                    st[:, qoff:qoff + P], triW[:],
                                    ident[:], start=False, stop=True)
                            pexp = pexp_pool.tile([P, 512], BF, tag="pexp")
                            nc.scalar.activation(
                                pexp[:, qoff:], st[:, qoff:], EXP,
                                scale=SM)
                            pending.append((b, qt, h, a, qoff, nk,
                                            pexp, at_ps, den_ps))
                            if len(pending) == 4:
                                flush(pending.pop(0))

                def normalize_group(b, qt):
                    """broadcast the reciprocals, scale, store to ATn"""
                    st_ = grp_state[(b, qt)]
                    cs = slice(S * b + 512 * qt, S * b + 512 * (qt + 1))
                    for h in range(NHL):
                        rden = rden_pool.tile([P, 512], F32, tag="rden")
                        nc.gpsimd.partition_broadcast(rden[:],
                                                      st_["rec"][h][:])
                        nc.vector.tensor_mul(
                            out=ATn[h][:, cs],
                            in0=st_["araw"][h][:], in1=rden[:])

                def oproj_group(b, qt):
                    """o_proj for the 4 token tiles of one (b, qt) group"""
                    for t16 in range(8 * b + 4 * qt, 8 * b + 4 * qt + 4):
                        ob = outb_pool.tile([P, HD], BF, tag="ob")
                        for ot in range(HD // 512):
                            ps = op_psum.tile([P, 512], F32, tag="op")
                            for j in range(NHL):
                                nc.tensor.matmul(
                                    ps[:], ATn[j][:, P * t16:P * (t16 + 1)],
                                    wo_sb[:, j, 512 * ot:512 * (ot + 1)],
                                    start=(j == 0), stop=(j == NHL - 1))
                            nc.vector.tensor_copy(
                                ob[:, 512 * ot:512 * (ot + 1)], ps[:])
                            if ot == 3:
                                nc.sync.dma_start(
                                    out[P * t16:P * (t16 + 1), 0:HD // 2],
                                    ob[:, 0:HD // 2])
                        nc.sync.dma_start(
                            out[P * t16:P * (t16 + 1), HD // 2:], ob[:, HD // 2:])

                groups = [(b, qt) for b in range(B) for qt in range(S // 512)]
                for gi, (b, qt) in enumerate(groups):
                    attn_group(b, qt)
                    # group gi-1's last evacuation lands during this
                    # group's first flushes, so its normalize can issue now
                    if gi >= 1:
                        normalize_group(*groups[gi - 1])
                    if gi >= 2:
                        oproj_group(*groups[gi - 2])
                while pending:
                    flush(pending.pop(0))
                normalize_group(*groups[-1])
                oproj_group(*groups[-2])
                oproj_group(*groups[-1])

    nc.compile()
    return nc


_NC = None


def _get_nc():
    global _NC
    if _NC is None:
        _NC = build_nc()
    return _NC


def make_in_maps(hidden_states, cos, sin, wq, wk, wv, wo):
    bf16 = ml_dtypes.bfloat16
    hidden_states = np.asarray(hidden_states, np.float32)
    cos = np.asarray(cos, np.float32)
    sin = np.asarray(sin, np.float32)
    wq = np.asarray(wq, np.float32)
    wk = np.asarray(wk, np.float32)
    wv = np.asarray(wv, np.float32)
    wo = np.asarray(wo, np.float32)

    HT = np.ascontiguousarray(hidden_states.T.astype(bf16))
    cosT = cos.T
    sinT = sin.T
    cosF = np.ascontiguousarray(np.concatenate([cosT, cosT], 0))
    sinF = np.ascontiguousarray(np.concatenate([-sinT, sinT], 0))

    in_maps = []
    for c in range(N_CORES):
        wq_c = wq[NHL * P * c:NHL * P * (c + 1)]
        wk_c = wk[P * c:P * (c + 1)]
        wv_c = wv[P * c:P * (c + 1)]
        wqkvT = np.ascontiguousarray(
            np.concatenate([wq_c, wk_c, wv_c], 0).T.astype(bf16))
        woT = np.ascontiguousarray(
            wo[:, NHL * P * c:NHL * P * (c + 1)].T.astype(bf16))
        in_maps.append(dict(hiddenT=HT, wqkvT=wqkvT, woT=woT,
                            cosF=cosF, sinF=sinF))
    return in_maps


def kernel(hidden_states, cos, sin, wq, wk, wv, wo, batch, seq_len):
    assert int(batch) == B and int(seq_len) == S
    nc = _get_nc()
    in_maps = make_in_maps(hidden_states, cos, sin, wq, wk, wv, wo)
    res = run_bass_kernel_spmd(nc, in_maps, core_ids=list(range(N_CORES)))
    acc = res.results[0]["out"].astype(np.float32)
    for c in range(1, N_CORES):
        acc += res.results[c]["out"].astype(np.float32)
    return acc


# revision 47
# speedup vs baseline: 1.0073x; 1.0073x over previous
"""Tensor-parallel FlashLlamaAttention kernel for 8 Trainium2 NeuronCores.

Sharding: each core owns 4 query heads (512 proj dims) and 1 kv head
(128 dims). Per-core device program computes qkv projection (+RoPE),
causal GQA attention and its o_proj partial product; the 8 partial
[2048, 4096] outputs are summed on the host (replaces the all-reduce).

Device-side layouts are all "feature on partitions" (transposed), so the
host wrapper pre-transposes hidden_states and the weight shards.
All matmul operands are bf16 (f32 PSUM accumulation): bf16 enables the
PE fast-weight-load path, so LDWEIGHTS hides behind the matmul stream
instead of serializing with it (f32 weights load at 1 elem/cycle and
cost ~180ns per 128x128 tile). Rotate-half for RoPE runs on the PE as a
permutation matmul (DVE cannot move data across partitions).

Softmax denominators are batched per (batch, q-tile) into a [4, 512]
tile and inverted with one reciprocal_approx_fast (the plain DVE
reciprocal costs ~3.3us per call); attention PSUM is evacuated with
plain copies so the normalization chain stays off the PE critical path.
"""
import sys

sys.path.insert(0, "/opt/trn_rl_repo")

from contextlib import ExitStack

import numpy as np
import ml_dtypes

import concourse.bass as bass
import concourse.bacc as bacc
import concourse.mybir as mybir
import concourse.tile as tile
from concourse.bass_utils import run_bass_kernel_spmd
from concourse.masks import make_identity

F32 = mybir.dt.float32
BF = mybir.dt.bfloat16
EXP = mybir.ActivationFunctionType.Exp

P = 128          # partitions / head dim
T = 2048         # total tokens (B * S)
S = 1024         # seq len per batch
B = 2
HD = 4096        # hidden dim
NHL = 4          # local query heads per core
DQKV = NHL * P + P + P  # 768 local projection dims (4q + k + v)
SM = float(P) ** -0.5

N_CORES = 8


def build_nc():
    nc = bacc.Bacc("TRN2", target_bir_lowering=False, debug=False,
                   num_devices=N_CORES)

    hiddenT = nc.dram_tensor("hiddenT", [HD, T], BF, kind="ExternalInput").ap()
    wqkvT = nc.dram_tensor("wqkvT", [HD, DQKV], BF, kind="ExternalInput").ap()
    woT = nc.dram_tensor("woT", [NHL * P, HD], BF, kind="ExternalInput").ap()
    cosF = nc.dram_tensor("cosF", [P, T], F32, kind="ExternalInput").ap()
    sinF = nc.dram_tensor("sinF", [P, T], F32, kind="ExternalInput").ap()
    out = nc.dram_tensor("out", [T, HD], BF, kind="ExternalOutput").ap()

    with tile.TileContext(nc) as tc, ExitStack() as stack:
        const = stack.enter_context(tc.tile_pool(name="const", bufs=1))
        ident_f = const.tile([P, P], F32)
        make_identity(nc, ident_f[:])
        ident = const.tile([P, P], BF)
        nc.vector.tensor_copy(ident[:], ident_f[:])
        # rotate-half permutation: perm[k, i] = 1 iff |k - i| == 64
        perm = const.tile([P, P], BF)
        nc.vector.tensor_copy(perm[:, 0:64], ident[:, 64:128])
        nc.vector.tensor_copy(perm[:, 64:128], ident[:, 0:64])
        ones_f32 = const.tile([P, 1], F32)
        nc.vector.memset(ones_f32[:], 1.0)
        ones_k = const.tile([P, 1], BF)
        nc.vector.tensor_copy(ones_k[:], ones_f32[:])
        # causal additive mask weight: triW[p, m] = -1e9 iff m > p.
        # Accumulated into the diagonal score tiles on the PE via
        # matmul(st, triW, ident) so the DVE never touches pexp.
        tri_f = const.tile([P, P], F32)
        nc.gpsimd.memset(tri_f[:], -1e9)
        nc.gpsimd.affine_select(
            out=tri_f[:], in_=tri_f[:], compare_op=mybir.AluOpType.is_ge,
            fill=0.0, base=-1, pattern=[[1, P]], channel_multiplier=-1)
        triW = const.tile([P, P], BF)
        nc.vector.tensor_copy(triW[:], tri_f[:])

        # long-lived activations, split per batch so attention on batch 0
        # never waits (tile-granular deps) on batch 1's rope/transpose work
        qkv_pool = stack.enter_context(tc.tile_pool(name="qkv", bufs=1))
        qT = [[qkv_pool.tile([P, S], BF, tag=f"qT{h}_{b}", name=f"qT{h}_{b}")
               for b in range(B)] for h in range(NHL)]
        kT = [qkv_pool.tile([P, S], BF, tag=f"kT{b}", name=f"kT{b}")
              for b in range(B)]
        v_pool = stack.enter_context(tc.tile_pool(name="v", bufs=1))
        v_sb = [v_pool.tile([P, S // P, P], BF, tag=f"v_sb{b}",
                            name=f"v_sb{b}") for b in range(B)]

        # o_proj weights tile; DMA issued inside phase 1 after the
        # wqkv/cos/sin loads so it drains during phase 1 without
        # delaying the first projection matmuls
        w2_pool = stack.enter_context(tc.tile_pool(name="w2", bufs=1))
        wo_sb = w2_pool.tile([P, NHL, HD], BF)
        woT_r = woT.rearrange("(a p) o -> p a o", p=P)

        # ---------------- phase 1: qkv projection + rope -----------------
        with (
            tc.tile_pool(name="cs", bufs=1) as cs_pool,
            tc.tile_pool(name="w1", bufs=1) as w1_pool,
            tc.tile_pool(name="xt", bufs=12) as xt_pool,
            tc.tile_pool(name="rot", bufs=2) as rot_pool,
            tc.tile_pool(name="qraw", bufs=5) as qraw_pool,
            tc.tile_pool(name="vtmp", bufs=1) as vtmp_pool,
            tc.tile_pool(name="pps", bufs=1, space="PSUM") as proj_psum,
            tc.tile_pool(name="rps", bufs=2, space="PSUM") as rope_psum,
        ):
            # one tile per 4-ho weight group: the first matmuls only
            # depend on the first group's DMA, not the whole 6 MB load
            # (deps are tile-granular)
            wqkvT_r = wqkvT.rearrange("(a p) j -> p a j", p=P)
            wqkv_g = []
            for g in range(8):
                wg = w1_pool.tile([P, 4, DQKV], BF, tag=f"wq{g}",
                                  name=f"wq{g}")
                nc.scalar.dma_start(wg[:], wqkvT_r[:, 4 * g:4 * (g + 1), :])
                wqkv_g.append(wg)
            cos_sb = cs_pool.tile([P, T], F32)
            sin_sb = cs_pool.tile([P, T], F32)
            nc.scalar.dma_start(cos_sb[:], cosF[:])
            nc.scalar.dma_start(sin_sb[:], sinF[:])
            vT_tmp = vtmp_pool.tile([P, T], BF, tag="vT", name="vT_tmp")

            for tc4 in range(T // 512):
                ts = slice(512 * tc4, 512 * (tc4 + 1))
                cb = tc4 // 2          # batch this chunk belongs to
                lts = slice(512 * (tc4 % 2), 512 * (tc4 % 2 + 1))
                dsts = [(h, qT[h][cb]) for h in range(NHL)] + [(4, kT[cb])]
                ps = [proj_psum.tile([P, 512], F32, tag=f"pps{ot}",
                                     name=f"pps{ot}_{tc4}")
                      for ot in range(6)]
                for ho in range(HD // P):
                    xt = xt_pool.tile([P, 512], BF, tag="xt")
                    nc.sync.dma_start(xt[:], hiddenT[P * ho:P * (ho + 1), ts])
                    for ot in range(6):
                        nc.tensor.matmul(
                            ps[ot][:],
                            wqkv_g[ho // 4][:, ho % 4, P * ot:P * (ot + 1)],
                            xt[:], start=(ho == 0), stop=(ho == HD // P - 1))
                # evacuate all 6 PSUM banks first (qraw copy + cos-mult
                # are each bank's only readers), so the next chunk's
                # matmuls unblock ASAP; the rot/sin/add RoPE work runs
                # afterwards off-PSUM
                nc.vector.tensor_copy(vT_tmp[:, ts], ps[5][:])
                qraws = []
                for idx, dst in dsts:
                    qraw = qraw_pool.tile([P, 512], BF, tag="qraw",
                                          name=f"qraw{idx}_{tc4}")
                    nc.scalar.copy(qraw[:], ps[idx][:])
                    qraws.append(qraw)
                    nc.vector.tensor_mul(out=dst[:, lts], in0=ps[idx][:],
                                         in1=cos_sb[:, ts])
                for qraw, (idx, dst) in zip(qraws, dsts):
                    rot_ps = rope_psum.tile([P, 512], F32, tag="rotp")
                    nc.tensor.matmul(rot_ps[:], perm[:], qraw[:],
                                     start=True, stop=True)
                    rt = rot_pool.tile([P, 512], F32, tag="rot")
                    nc.vector.tensor_mul(out=rt[:], in0=rot_ps[:],
                                         in1=sin_sb[:, ts])
                    nc.vector.tensor_add(out=dst[:, lts], in0=dst[:, lts],
                                         in1=rt[:])
                # transpose this chunk of V: vT [j, t] -> v_sb [t, tchunk, j]
                # via DMA xbar transpose -- keeps it off the PE/DVE/ACT
                # engines and off the rope PSUM banks entirely
                for tt in range(4 * tc4, 4 * (tc4 + 1)):
                    nc.scalar.dma_start(v_sb[cb][:, tt - 8 * cb, :],
                                        vT_tmp[:, P * tt:P * (tt + 1)],
                                        transpose=True)

        # ----- long-lived attention output (allocated after phase 1 frees)
        at_pool = stack.enter_context(tc.tile_pool(name="at", bufs=1))
        ATn = [at_pool.tile([P, T], BF, tag=f"ATn{h}", name=f"ATn{h}")
               for h in range(NHL)]
        outb_pool = stack.enter_context(tc.tile_pool(name="ob", bufs=2))

        # o_proj weights load at phase-2 start on the sync queue (idle by
        # now): the 8MB of HBM traffic lands during attention, well
        # before the first oproj group needs it
        for g in range(NHL):
            nc.sync.dma_start(wo_sb[:, g, :], woT_r[:, g, :])

        # ---------------- phase 2: causal GQA attention --------------
        if True:
            with (
                tc.tile_pool(name="pexp", bufs=6) as pexp_pool,
                tc.tile_pool(name="araw", bufs=8) as araw_pool,
                tc.tile_pool(name="dens", bufs=8) as den_small,
                tc.tile_pool(name="rden", bufs=3) as rden_pool,
                tc.tile_pool(name="stp", bufs=2, space="PSUM") as st_psum,
                tc.tile_pool(name="atp", bufs=2, space="PSUM") as at_psum,
                tc.tile_pool(name="dnp", bufs=2, space="PSUM") as den_psum,
                tc.tile_pool(name="opp", bufs=2, space="PSUM") as op_psum,
            ):
                # per (b, qt) group state: at_raw tiles + den reciprocals
                grp_state = {}

                def evac_group(b, qt, h, at_ps, den_ps):
                    """copy one finished (b, qt, h) accumulation to SBUF"""
                    st_ = grp_state[(b, qt)]
                    araw = araw_pool.tile([P, 512], F32, tag="araw",
                                          name=f"araw{b}_{qt}_{h}")
                    nc.scalar.copy(araw[:], at_ps[:])
                    den_c = den_small.tile([1, 512], F32, tag="den",
                                           name=f"den{b}_{qt}_{h}")
                    nc.scalar.copy(den_c[:], den_ps[:])
                    rec = den_small.tile([1, 512], F32, tag="rec",
                                         name=f"rec{b}_{qt}_{h}")
                    nc.vector.reciprocal_approx_fast(rec[:], den_c[:])
                    st_["araw"][h] = araw
                    st_["rec"][h] = rec

                def flush(item):
                    b, qt, h, a0, qo0, nk, px0, at_ps, den_ps = item
                    nc.tensor.matmul(
                        at_ps[:, qo0:], v_sb[b][:, a0, :],
                        px0[:, qo0:], start=(a0 == 0), stop=(a0 == nk - 1))
                    nc.tensor.matmul(
                        den_ps[:, qo0:], ones_k[:],
                        px0[:, qo0:], start=(a0 == 0), stop=(a0 == nk - 1))
                    if a0 == nk - 1:
                        evac_group(b, qt, h, at_ps, den_ps)

                pending = []       # score tiles awaiting their AV/den matmul

                def attn_group(b, qt):
                    """issue the 4 heads' score/exp/AV/den for one q tile"""
                    grp_state[(b, qt)] = {"araw": {}, "rec": {}}
                    nk = 4 * qt + 4
                    for h in range(NHL):
                        qTb = qT[h][b][:]
                        kTb = kT[b][:]
                        at_ps = at_psum.tile([P, 512], F32, tag="at")
                        den_ps = den_psum.tile([1, 512], F32, tag="den")
                        for a in range(nk):
                            qoff = max(0, P * a - 512 * qt)
                            diag = P * a >= 512 * qt
                            st = st_psum.tile([P, 512], F32, tag="st")
                            nc.tensor.matmul(
                                st[:, qoff:],
                                kTb[:, P * a:P * (a + 1)],
                                qTb[:, 512 * qt + qoff:512 * (qt + 1)],
                                start=True, stop=not diag)
                            if diag:
                                # st[k, q] += -1e9 for k > q in the
                                # diagonal 128x128 block
                                nc.tensor.matmul(
                                    st[:, qoff:qoff + P], triW[:],
                                    ident[:], start=False, stop=True)
                            pexp = pexp_pool.tile([P, 512], BF, tag="pexp")
                            nc.scalar.activation(
                                pexp[:, qoff:], st[:, qoff:], EXP,
                                scale=SM)
                            pending.append((b, qt, h, a, qoff, nk,
                                            pexp, at_ps, den_ps))
                            if len(pending) == 4:
                                flush(pending.pop(0))

                def normalize_group(b, qt):
                    """broadcast the reciprocals, scale, store to ATn"""
                    st_ = grp_state[(b, qt)]
                    cs = slice(S * b + 512 * qt, S * b + 512 * (qt + 1))
                    for h in range(NHL):
                        rden = rden_pool.tile([P, 512], F32, tag="rden")
                        nc.gpsimd.partition_broadcast(rden[:],
                                                      st_["rec"][h][:])
                        nc.vector.tensor_mul(
                            out=ATn[h][:, cs],
                            in0=st_["araw"][h][:], in1=rden[:])

                def oproj_group(b, qt):
                    """o_proj for the 4 token tiles of one (b, qt) group"""
                    for t16 in range(8 * b + 4 * qt, 8 * b + 4 * qt + 4):
                        ob = outb_pool.tile([P, HD], BF, tag="ob")
                        for ot in range(HD // 512):
                            ps = op_psum.tile([P, 512], F32, tag="op")
                            for j in range(NHL):
                                nc.tensor.matmul(
                                    ps[:], ATn[j][:, P * t16:P * (t16 + 1)],
                                    wo_sb[:, j, 512 * ot:512 * (ot + 1)],
                                    start=(j == 0), stop=(j == NHL - 1))
                            nc.vector.tensor_copy(
                                ob[:, 512 * ot:512 * (ot + 1)], ps[:])
                            if ot == 3:
                                nc.sync.dma_start(
                                    out[P * t16:P * (t16 + 1), 0:HD // 2],
                                    ob[:, 0:HD // 2])
                        nc.sync.dma_start(
                            out[P * t16:P * (t16 + 1), HD // 2:], ob[:, HD // 2:])

                groups = [(b, qt) for b in range(B) for qt in range(S // 512)]
                for gi, (b, qt) in enumerate(groups):
                    attn_group(b, qt)
                    # group gi-1's last evacuation lands during this
                    # group's first flushes, so its normalize can issue now
                    if gi >= 1:
                        normalize_group(*groups[gi - 1])
                    if gi >= 2:
                        oproj_group(*groups[gi - 2])
                while pending:
                    flush(pending.pop(0))
                normalize_group(*groups[-1])
                oproj_group(*groups[-2])
                oproj_group(*groups[-1])

    nc.compile()
    return nc


_NC = None


def _get_nc():
    global _NC
    if _NC is None:
        _NC = build_nc()
    return _NC


def make_in_maps(hidden_states, cos, sin, wq, wk, wv, wo):
    bf16 = ml_dtypes.bfloat16
    hidden_states = np.asarray(hidden_states, np.float32)
    cos = np.asarray(cos, np.float32)
    sin = np.asarray(sin, np.float32)
    wq = np.asarray(wq, np.float32)
    wk = np.asarray(wk, np.float32)
    wv = np.asarray(wv, np.float32)
    wo = np.asarray(wo, np.float32)

    HT = np.ascontiguousarray(hidden_states.T.astype(bf16))
    cosT = cos.T
    sinT = sin.T
    cosF = np.ascontiguousarray(np.concatenate([cosT, cosT], 0))
    sinF = np.ascontiguousarray(np.concatenate([-sinT, sinT], 0))

    in_maps = []
    for c in range(N_CORES):
        wq_c = wq[NHL * P * c:NHL * P * (c + 1)]
        wk_c = wk[P * c:P * (c + 1)]
        wv_c = wv[P * c:P * (c + 1)]
        wqkvT = np.ascontiguousarray(
            np.concatenate([wq_c, wk_c, wv_c], 0).T.astype(bf16))
        woT = np.ascontiguousarray(
            wo[:, NHL * P * c:NHL * P * (c + 1)].T.astype(bf16))
        in_maps.append(dict(hiddenT=HT, wqkvT=wqkvT, woT=woT,
                            cosF=cosF, sinF=sinF))
    return in_maps


def kernel(hidden_states, cos, sin, wq, wk, wv, wo, batch, seq_len):
    assert int(batch) == B and int(seq_len) == S
    nc = _get_nc()
    in_maps = make_in_maps(hidden_states, cos, sin, wq, wk, wv, wo)
    res = run_bass_kernel_spmd(nc, in_maps, core_ids=list(range(N_CORES)))
    acc = res.results[0]["out"].astype(np.float32)
    for c in range(1, N_CORES):
        acc += res.results[c]["out"].astype(np.float32)
    return acc


# revision 51
# speedup vs baseline: 1.0228x; 1.0154x over previous
"""Tensor-parallel FlashLlamaAttention kernel for 8 Trainium2 NeuronCores.

Sharding: each core owns 4 query heads (512 proj dims) and 1 kv head
(128 dims). Per-core device program computes qkv projection (+RoPE),
causal GQA attention and its o_proj partial product; the 8 partial
[2048, 4096] outputs are summed on the host (replaces the all-reduce).

Device-side layouts are all "feature on partitions" (transposed), so the
host wrapper pre-transposes hidden_states and the weight shards.
All matmul operands are bf16 (f32 PSUM accumulation): bf16 enables the
PE fast-weight-load path, so LDWEIGHTS hides behind the matmul stream
instead of serializing with it (f32 weights load at 1 elem/cycle and
cost ~180ns per 128x128 tile). Rotate-half for RoPE runs on the PE as a
permutation matmul (DVE cannot move data across partitions).

Softmax denominators are batched per (batch, q-tile) into a [4, 512]
tile and inverted with one reciprocal_approx_fast (the plain DVE
reciprocal costs ~3.3us per call); attention PSUM is evacuated with
plain copies so the normalization chain stays off the PE critical path.
"""
import sys

sys.path.insert(0, "/opt/trn_rl_repo")

from contextlib import ExitStack

import numpy as np
import ml_dtypes

import concourse.bass as bass
import concourse.bacc as bacc
import concourse.mybir as mybir
import concourse.tile as tile
from concourse.bass_utils import run_bass_kernel_spmd
from concourse.masks import make_identity

F32 = mybir.dt.float32
BF = mybir.dt.bfloat16
EXP = mybir.ActivationFunctionType.Exp

P = 128          # partitions / head dim
T = 2048         # total tokens (B * S)
S = 1024         # seq len per batch
B = 2
HD = 4096        # hidden dim
NHL = 4          # local query heads per core
DQKV = NHL * P + P + P  # 768 local projection dims (4q + k + v)
SM = float(P) ** -0.5

N_CORES = 8


def build_nc():
    nc = bacc.Bacc("TRN2", target_bir_lowering=False, debug=False,
                   num_devices=N_CORES)

    hiddenT = nc.dram_tensor("hiddenT", [HD, T], BF, kind="ExternalInput").ap()
    wqkvT = nc.dram_tensor("wqkvT", [HD, DQKV], BF, kind="ExternalInput").ap()
    woT = nc.dram_tensor("woT", [NHL * P, HD], BF, kind="ExternalInput").ap()
    cosF = nc.dram_tensor("cosF", [P, T], F32, kind="ExternalInput").ap()
    sinF = nc.dram_tensor("sinF", [P, T], F32, kind="ExternalInput").ap()
    out = nc.dram_tensor("out", [T, HD], BF, kind="ExternalOutput").ap()

    with tile.TileContext(nc) as tc, ExitStack() as stack:
        const = stack.enter_context(tc.tile_pool(name="const", bufs=1))
        ident_f = const.tile([P, P], F32)
        make_identity(nc, ident_f[:])
        ident = const.tile([P, P], BF)
        nc.vector.tensor_copy(ident[:], ident_f[:])
        # rotate-half permutation: perm[k, i] = 1 iff |k - i| == 64
        perm = const.tile([P, P], BF)
        nc.vector.tensor_copy(perm[:, 0:64], ident[:, 64:128])
        nc.vector.tensor_copy(perm[:, 64:128], ident[:, 0:64])
        ones_f32 = const.tile([P, 1], F32)
        nc.vector.memset(ones_f32[:], 1.0)
        ones_k = const.tile([P, 1], BF)
        nc.vector.tensor_copy(ones_k[:], ones_f32[:])
        # causal additive mask weight: triW[p, m] = -1e9 iff m > p.
        # Accumulated into the diagonal score tiles on the PE via
        # matmul(st, triW, ident) so the DVE never touches pexp.
        tri_f = const.tile([P, P], F32)
        nc.gpsimd.memset(tri_f[:], -1e9)
        nc.gpsimd.affine_select(
            out=tri_f[:], in_=tri_f[:], compare_op=mybir.AluOpType.is_ge,
            fill=0.0, base=-1, pattern=[[1, P]], channel_multiplier=-1)
        triW = const.tile([P, P], BF)
        nc.vector.tensor_copy(triW[:], tri_f[:])

        # long-lived activations, split per batch so attention on batch 0
        # never waits (tile-granular deps) on batch 1's rope/transpose work
        qkv_pool = stack.enter_context(tc.tile_pool(name="qkv", bufs=1))
        qT = [[qkv_pool.tile([P, S], BF, tag=f"qT{h}_{b}", name=f"qT{h}_{b}")
               for b in range(B)] for h in range(NHL)]
        kT = [qkv_pool.tile([P, S], BF, tag=f"kT{b}", name=f"kT{b}")
              for b in range(B)]
        v_pool = stack.enter_context(tc.tile_pool(name="v", bufs=1))
        v_sb = [v_pool.tile([P, S // P, P], BF, tag=f"v_sb{b}",
                            name=f"v_sb{b}") for b in range(B)]

        # o_proj weights tile; DMA issued inside phase 1 after the
        # wqkv/cos/sin loads so it drains during phase 1 without
        # delaying the first projection matmuls
        w2_pool = stack.enter_context(tc.tile_pool(name="w2", bufs=1))
        wo_sb = w2_pool.tile([P, NHL, HD], BF)
        woT_r = woT.rearrange("(a p) o -> p a o", p=P)

        # ---------------- phase 1: qkv projection + rope -----------------
        with (
            tc.tile_pool(name="cs", bufs=1) as cs_pool,
            tc.tile_pool(name="w1", bufs=1) as w1_pool,
            tc.tile_pool(name="xt", bufs=12) as xt_pool,
            tc.tile_pool(name="rot", bufs=2) as rot_pool,
            tc.tile_pool(name="qraw", bufs=5) as qraw_pool,
            tc.tile_pool(name="vtmp", bufs=1) as vtmp_pool,
            tc.tile_pool(name="pps", bufs=1, space="PSUM") as proj_psum,
            tc.tile_pool(name="rps", bufs=2, space="PSUM") as rope_psum,
        ):
            # one tile per 4-ho weight group: the first matmuls only
            # depend on the first group's DMA, not the whole 6 MB load
            # (deps are tile-granular)
            wqkvT_r = wqkvT.rearrange("(a p) j -> p a j", p=P)
            wqkv_g = []
            for g in range(8):
                wg = w1_pool.tile([P, 4, DQKV], BF, tag=f"wq{g}",
                                  name=f"wq{g}")
                nc.scalar.dma_start(wg[:], wqkvT_r[:, 4 * g:4 * (g + 1), :])
                wqkv_g.append(wg)
            cos_sb = cs_pool.tile([P, T], F32)
            sin_sb = cs_pool.tile([P, T], F32)
            nc.scalar.dma_start(cos_sb[:], cosF[:])
            nc.scalar.dma_start(sin_sb[:], sinF[:])
            vT_tmp = vtmp_pool.tile([P, T], BF, tag="vT", name="vT_tmp")

            for tc4 in range(T // 512):
                ts = slice(512 * tc4, 512 * (tc4 + 1))
                cb = tc4 // 2          # batch this chunk belongs to
                lts = slice(512 * (tc4 % 2), 512 * (tc4 % 2 + 1))
                dsts = [(h, qT[h][cb]) for h in range(NHL)] + [(4, kT[cb])]
                ps = [proj_psum.tile([P, 512], F32, tag=f"pps{ot}",
                                     name=f"pps{ot}_{tc4}")
                      for ot in range(6)]
                for ho in range(HD // P):
                    xt = xt_pool.tile([P, 512], BF, tag="xt")
                    nc.sync.dma_start(xt[:], hiddenT[P * ho:P * (ho + 1), ts])
                    for ot in range(6):
                        nc.tensor.matmul(
                            ps[ot][:],
                            wqkv_g[ho // 4][:, ho % 4, P * ot:P * (ot + 1)],
                            xt[:], start=(ho == 0), stop=(ho == HD // P - 1))
                # evacuate all 6 PSUM banks first (qraw copy + cos-mult
                # are each bank's only readers), so the next chunk's
                # matmuls unblock ASAP; the rot/sin/add RoPE work runs
                # afterwards off-PSUM
                nc.vector.tensor_copy(vT_tmp[:, ts], ps[5][:])
                qraws = []
                for idx, dst in dsts:
                    qraw = qraw_pool.tile([P, 512], BF, tag="qraw",
                                          name=f"qraw{idx}_{tc4}")
                    nc.scalar.copy(qraw[:], ps[idx][:])
                    qraws.append(qraw)
                    nc.vector.tensor_mul(out=dst[:, lts], in0=ps[idx][:],
                                         in1=cos_sb[:, ts])
                for qraw, (idx, dst) in zip(qraws, dsts):
                    rot_ps = rope_psum.tile([P, 512], F32, tag="rotp")
                    nc.tensor.matmul(rot_ps[:], perm[:], qraw[:],
                                     start=True, stop=True)
                    rt = rot_pool.tile([P, 512], F32, tag="rot")
                    nc.vector.tensor_mul(out=rt[:], in0=rot_ps[:],
                                         in1=sin_sb[:, ts])
                    nc.vector.tensor_add(out=dst[:, lts], in0=dst[:, lts],
                                         in1=rt[:])
                # transpose this chunk of V: vT [j, t] -> v_sb [t, tchunk, j]
                # via DMA xbar transpose -- keeps it off the PE/DVE/ACT
                # engines and off the rope PSUM banks entirely
                for tt in range(4 * tc4, 4 * (tc4 + 1)):
                    nc.scalar.dma_start(v_sb[cb][:, tt - 8 * cb, :],
                                        vT_tmp[:, P * tt:P * (tt + 1)],
                                        transpose=True)

        # ----- long-lived attention output (allocated after phase 1 frees)
        at_pool = stack.enter_context(tc.tile_pool(name="at", bufs=1))
        ATn = [at_pool.tile([P, T], BF, tag=f"ATn{h}", name=f"ATn{h}")
               for h in range(NHL)]
        outb_pool = stack.enter_context(tc.tile_pool(name="ob", bufs=2))

        # o_proj weights load at phase-2 start on the sync queue (idle by
        # now): the 8MB of HBM traffic lands during attention, well
        # before the first oproj group needs it
        for g in range(NHL):
            nc.sync.dma_start(wo_sb[:, g, :], woT_r[:, g, :])

        # ---------------- phase 2: causal GQA attention --------------
        if True:
            with (
                tc.tile_pool(name="pexp", bufs=8) as pexp_pool,
                tc.tile_pool(name="araw", bufs=8) as araw_pool,
                tc.tile_pool(name="dens", bufs=8) as den_small,
                tc.tile_pool(name="rden", bufs=3) as rden_pool,
                tc.tile_pool(name="stp", bufs=3, space="PSUM") as st_psum,
                tc.tile_pool(name="atp", bufs=2, space="PSUM") as at_psum,
                tc.tile_pool(name="dnp", bufs=1, space="PSUM") as den_psum,
                tc.tile_pool(name="opp", bufs=2, space="PSUM") as op_psum,
            ):
                # per (b, qt) group state: at_raw tiles + den reciprocals
                grp_state = {}

                def evac_group(b, qt, h, at_ps, den_ps):
                    """copy one finished (b, qt, h) accumulation to SBUF"""
                    st_ = grp_state[(b, qt)]
                    araw = araw_pool.tile([P, 512], F32, tag="araw",
                                          name=f"araw{b}_{qt}_{h}")
                    nc.scalar.copy(araw[:], at_ps[:])
                    den_c = den_small.tile([1, 512], F32, tag="den",
                                           name=f"den{b}_{qt}_{h}")
                    nc.scalar.copy(den_c[:], den_ps[:])
                    rec = den_small.tile([1, 512], F32, tag="rec",
                                         name=f"rec{b}_{qt}_{h}")
                    nc.vector.reciprocal_approx_fast(rec[:], den_c[:])
                    st_["araw"][h] = araw
                    st_["rec"][h] = rec

                def flush(item):
                    b, qt, h, a0, qo0, nk, px0, at_ps, den_ps = item
                    nc.tensor.matmul(
                        at_ps[:, qo0:], v_sb[b][:, a0, :],
                        px0[:, qo0:], start=(a0 == 0), stop=(a0 == nk - 1))
                    nc.tensor.matmul(
                        den_ps[:, qo0:], ones_k[:],
                        px0[:, qo0:], start=(a0 == 0), stop=(a0 == nk - 1))
                    if a0 == nk - 1:
                        evac_group(b, qt, h, at_ps, den_ps)

                pending = []       # score tiles awaiting their AV/den matmul

                def attn_group(b, qt):
                    """issue the 4 heads' score/exp/AV/den for one q tile"""
                    grp_state[(b, qt)] = {"araw": {}, "rec": {}}
                    nk = 4 * qt + 4
                    for h in range(NHL):
                        qTb = qT[h][b][:]
                        kTb = kT[b][:]
                        at_ps = at_psum.tile([P, 512], F32, tag="at")
                        den_ps = den_psum.tile([1, 512], F32, tag="den")
                        for a in range(nk):
                            qoff = max(0, P * a - 512 * qt)
                            diag = P * a >= 512 * qt
                            st = st_psum.tile([P, 512], F32, tag="st")
                            nc.tensor.matmul(
                                st[:, qoff:],
                                kTb[:, P * a:P * (a + 1)],
                                qTb[:, 512 * qt + qoff:512 * (qt + 1)],
                                start=True, stop=not diag)
                            if diag:
                                # st[k, q] += -1e9 for k > q in the
                                # diagonal 128x128 block
                                nc.tensor.matmul(
                                    st[:, qoff:qoff + P], triW[:],
                                    ident[:], start=False, stop=True)
                            pexp = pexp_pool.tile([P, 512], BF, tag="pexp")
                            nc.scalar.activation(
                                pexp[:, qoff:], st[:, qoff:], EXP,
                                scale=SM)
                            pending.append((b, qt, h, a, qoff, nk,
                                            pexp, at_ps, den_ps))
                            if len(pending) == 5:
                                flush(pending.pop(0))

                def normalize_group(b, qt):
                    """broadcast the reciprocals, scale, store to ATn"""
                    st_ = grp_state[(b, qt)]
                    cs = slice(S * b + 512 * qt, S * b + 512 * (qt + 1))
                    for h in range(NHL):
                        rden = rden_pool.tile([P, 512], F32, tag="rden")
                        nc.gpsimd.partition_broadcast(rden[:],
                                                      st_["rec"][h][:])
                        nc.vector.tensor_mul(
                            out=ATn[h][:, cs],
                            in0=st_["araw"][h][:], in1=rden[:])

                def oproj_group(b, qt):
                    """o_proj for the 4 token tiles of one (b, qt) group"""
                    for t16 in range(8 * b + 4 * qt, 8 * b + 4 * qt + 4):
                        ob = outb_pool.tile([P, HD], BF, tag="ob")
                        for ot in range(HD // 512):
                            ps = op_psum.tile([P, 512], F32, tag="op")
                            for j in range(NHL):
                                nc.tensor.matmul(
                                    ps[:], ATn[j][:, P * t16:P * (t16 + 1)],
                                    wo_sb[:, j, 512 * ot:512 * (ot + 1)],
                                    start=(j == 0), stop=(j == NHL - 1))
                            # alternate evacuation between DVE and ACT so
                            # neither FIFO's latency gates the two PSUM
                            # banks' turnover
                            if ot % 2 == 0:
                                nc.vector.tensor_copy(
                                    ob[:, 512 * ot:512 * (ot + 1)], ps[:])
                            else:
                                nc.scalar.copy(
                                    ob[:, 512 * ot:512 * (ot + 1)], ps[:])
                            if ot == 3:
                                nc.sync.dma_start(
                                    out[P * t16:P * (t16 + 1), 0:HD // 2],
                                    ob[:, 0:HD // 2])
                        nc.sync.dma_start(
                            out[P * t16:P * (t16 + 1), HD // 2:], ob[:, HD // 2:])

                groups = [(b, qt) for b in range(B) for qt in range(S // 512)]
                for gi, (b, qt) in enumerate(groups):
                    attn_group(b, qt)
                    # group gi-1's last evacuation lands during this
                    # group's first flushes, so its normalize can issue now
                    if gi >= 1:
                        normalize_group(*groups[gi - 1])
                    if gi >= 2:
                        oproj_group(*groups[gi - 2])
                while pending:
                    flush(pending.pop(0))
                normalize_group(*groups[-1])
                oproj_group(*groups[-2])
                oproj_group(*groups[-1])

    nc.compile()
    return nc


_NC = None


def _get_nc():
    global _NC
    if _NC is None:
        _NC = build_nc()
    return _NC


def make_in_maps(hidden_states, cos, sin, wq, wk, wv, wo):
    bf16 = ml_dtypes.bfloat16
    hidden_states = np.asarray(hidden_states, np.float32)
    cos = np.asarray(cos, np.float32)
    sin = np.asarray(sin, np.float32)
    wq = np.asarray(wq, np.float32)
    wk = np.asarray(wk, np.float32)
    wv = np.asarray(wv, np.float32)
    wo = np.asarray(wo, np.float32)

    HT = np.ascontiguousarray(hidden_states.T.astype(bf16))
    cosT = cos.T
    sinT = sin.T
    cosF = np.ascontiguousarray(np.concatenate([cosT, cosT], 0))
    sinF = np.ascontiguousarray(np.concatenate([-sinT, sinT], 0))

    in_maps = []
    for c in range(N_CORES):
        wq_c = wq[NHL * P * c:NHL * P * (c + 1)]
        wk_c = wk[P * c:P * (c + 1)]
        wv_c = wv[P * c:P * (c + 1)]
        wqkvT = np.ascontiguousarray(
            np.concatenate([wq_c, wk_c, wv_c], 0).T.astype(bf16))
        woT = np.ascontiguousarray(
            wo[:, NHL * P * c:NHL * P * (c + 1)].T.astype(bf16))
        in_maps.append(dict(hiddenT=HT, wqkvT=wqkvT, woT=woT,
                            cosF=cosF, sinF=sinF))
    return in_maps


def kernel(hidden_states, cos, sin, wq, wk, wv, wo, batch, seq_len):
    assert int(batch) == B and int(seq_len) == S
    nc = _get_nc()
    in_maps = make_in_maps(hidden_states, cos, sin, wq, wk, wv, wo)
    res = run_bass_kernel_spmd(nc, in_maps, core_ids=list(range(N_CORES)))
    acc = res.results[0]["out"].astype(np.float32)
    for c in range(1, N_CORES):
        acc += res.results[c]["out"].astype(np.float32)
    return acc


# revision 52
# speedup vs baseline: 1.0278x; 1.0049x over previous
"""Tensor-parallel FlashLlamaAttention kernel for 8 Trainium2 NeuronCores.

Sharding: each core owns 4 query heads (512 proj dims) and 1 kv head
(128 dims). Per-core device program computes qkv projection (+RoPE),
causal GQA attention and its o_proj partial product; the 8 partial
[2048, 4096] outputs are summed on the host (replaces the all-reduce).

Device-side layouts are all "feature on partitions" (transposed), so the
host wrapper pre-transposes hidden_states and the weight shards.
All matmul operands are bf16 (f32 PSUM accumulation): bf16 enables the
PE fast-weight-load path, so LDWEIGHTS hides behind the matmul stream
instead of serializing with it (f32 weights load at 1 elem/cycle and
cost ~180ns per 128x128 tile). Rotate-half for RoPE runs on the PE as a
permutation matmul (DVE cannot move data across partitions).

Softmax denominators are batched per (batch, q-tile) into a [4, 512]
tile and inverted with one reciprocal_approx_fast (the plain DVE
reciprocal costs ~3.3us per call); attention PSUM is evacuated with
plain copies so the normalization chain stays off the PE critical path.
"""
import sys

sys.path.insert(0, "/opt/trn_rl_repo")

from contextlib import ExitStack

import numpy as np
import ml_dtypes

import concourse.bass as bass
import concourse.bacc as bacc
import concourse.mybir as mybir
import concourse.tile as tile
from concourse.bass_utils import run_bass_kernel_spmd
from concourse.masks import make_identity

F32 = mybir.dt.float32
BF = mybir.dt.bfloat16
EXP = mybir.ActivationFunctionType.Exp

P = 128          # partitions / head dim
T = 2048         # total tokens (B * S)
S = 1024         # seq len per batch
B = 2
HD = 4096        # hidden dim
NHL = 4          # local query heads per core
DQKV = NHL * P + P + P  # 768 local projection dims (4q + k + v)
SM = float(P) ** -0.5

N_CORES = 8


def build_nc():
    nc = bacc.Bacc("TRN2", target_bir_lowering=False, debug=False,
                   num_devices=N_CORES)

    hiddenT = nc.dram_tensor("hiddenT", [HD, T], BF, kind="ExternalInput").ap()
    wqkvT = nc.dram_tensor("wqkvT", [HD, DQKV], BF, kind="ExternalInput").ap()
    woT = nc.dram_tensor("woT", [NHL * P, HD], BF, kind="ExternalInput").ap()
    cosF = nc.dram_tensor("cosF", [P, T], F32, kind="ExternalInput").ap()
    sinF = nc.dram_tensor("sinF", [P, T], F32, kind="ExternalInput").ap()
    out = nc.dram_tensor("out", [T, HD], BF, kind="ExternalOutput").ap()

    with tile.TileContext(nc) as tc, ExitStack() as stack:
        const = stack.enter_context(tc.tile_pool(name="const", bufs=1))
        ident_f = const.tile([P, P], F32)
        make_identity(nc, ident_f[:])
        ident = const.tile([P, P], BF)
        nc.vector.tensor_copy(ident[:], ident_f[:])
        # rotate-half permutation: perm[k, i] = 1 iff |k - i| == 64
        perm = const.tile([P, P], BF)
        nc.vector.tensor_copy(perm[:, 0:64], ident[:, 64:128])
        nc.vector.tensor_copy(perm[:, 64:128], ident[:, 0:64])
        ones_f32 = const.tile([P, 1], F32)
        nc.vector.memset(ones_f32[:], 1.0)
        ones_k = const.tile([P, 1], BF)
        nc.vector.tensor_copy(ones_k[:], ones_f32[:])
        # causal additive mask weight: triW[p, m] = -1e9 iff m > p.
        # Accumulated into the diagonal score tiles on the PE via
        # matmul(st, triW, ident) so the DVE never touches pexp.
        tri_f = const.tile([P, P], F32)
        nc.gpsimd.memset(tri_f[:], -1e9)
        nc.gpsimd.affine_select(
            out=tri_f[:], in_=tri_f[:], compare_op=mybir.AluOpType.is_ge,
            fill=0.0, base=-1, pattern=[[1, P]], channel_multiplier=-1)
        triW = const.tile([P, P], BF)
        nc.vector.tensor_copy(triW[:], tri_f[:])

        # long-lived activations, split per batch so attention on batch 0
        # never waits (tile-granular deps) on batch 1's rope/transpose work
        qkv_pool = stack.enter_context(tc.tile_pool(name="qkv", bufs=1))
        qT = [[qkv_pool.tile([P, S], BF, tag=f"qT{h}_{b}", name=f"qT{h}_{b}")
               for b in range(B)] for h in range(NHL)]
        kT = [qkv_pool.tile([P, S], BF, tag=f"kT{b}", name=f"kT{b}")
              for b in range(B)]
        v_pool = stack.enter_context(tc.tile_pool(name="v", bufs=1))
        v_sb = [v_pool.tile([P, S // P, P], BF, tag=f"v_sb{b}",
                            name=f"v_sb{b}") for b in range(B)]

        # o_proj weights tile; DMA issued inside phase 1 after the
        # wqkv/cos/sin loads so it drains during phase 1 without
        # delaying the first projection matmuls
        w2_pool = stack.enter_context(tc.tile_pool(name="w2", bufs=1))
        wo_sb = w2_pool.tile([P, NHL, HD], BF)
        woT_r = woT.rearrange("(a p) o -> p a o", p=P)

        # ---------------- phase 1: qkv projection + rope -----------------
        with (
            tc.tile_pool(name="cs", bufs=1) as cs_pool,
            tc.tile_pool(name="w1", bufs=1) as w1_pool,
            tc.tile_pool(name="xt", bufs=12) as xt_pool,
            tc.tile_pool(name="rot", bufs=2) as rot_pool,
            tc.tile_pool(name="qraw", bufs=5) as qraw_pool,
            tc.tile_pool(name="vtmp", bufs=1) as vtmp_pool,
            tc.tile_pool(name="pps", bufs=1, space="PSUM") as proj_psum,
            tc.tile_pool(name="rps", bufs=2, space="PSUM") as rope_psum,
        ):
            # one tile per 4-ho weight group: the first matmuls only
            # depend on the first group's DMA, not the whole 6 MB load
            # (deps are tile-granular)
            wqkvT_r = wqkvT.rearrange("(a p) j -> p a j", p=P)
            wqkv_g = []
            for g in range(8):
                wg = w1_pool.tile([P, 4, DQKV], BF, tag=f"wq{g}",
                                  name=f"wq{g}")
                nc.scalar.dma_start(wg[:], wqkvT_r[:, 4 * g:4 * (g + 1), :])
                wqkv_g.append(wg)
            cos_sb = cs_pool.tile([P, T], F32)
            sin_sb = cs_pool.tile([P, T], F32)
            nc.scalar.dma_start(cos_sb[:], cosF[:])
            nc.scalar.dma_start(sin_sb[:], sinF[:])
            vT_tmp = vtmp_pool.tile([P, T], BF, tag="vT", name="vT_tmp")

            for tc4 in range(T // 512):
                ts = slice(512 * tc4, 512 * (tc4 + 1))
                cb = tc4 // 2          # batch this chunk belongs to
                lts = slice(512 * (tc4 % 2), 512 * (tc4 % 2 + 1))
                dsts = [(h, qT[h][cb]) for h in range(NHL)] + [(4, kT[cb])]
                ps = [proj_psum.tile([P, 512], F32, tag=f"pps{ot}",
                                     name=f"pps{ot}_{tc4}")
                      for ot in range(6)]
                for ho in range(HD // P):
                    xt = xt_pool.tile([P, 512], BF, tag="xt")
                    nc.sync.dma_start(xt[:], hiddenT[P * ho:P * (ho + 1), ts])
                    for ot in range(6):
                        nc.tensor.matmul(
                            ps[ot][:],
                            wqkv_g[ho // 4][:, ho % 4, P * ot:P * (ot + 1)],
                            xt[:], start=(ho == 0), stop=(ho == HD // P - 1))
                # evacuate all 6 PSUM banks first (qraw copy + cos-mult
                # are each bank's only readers), so the next chunk's
                # matmuls unblock ASAP; the rot/sin/add RoPE work runs
                # afterwards off-PSUM
                nc.vector.tensor_copy(vT_tmp[:, ts], ps[5][:])
                qraws = []
                for idx, dst in dsts:
                    qraw = qraw_pool.tile([P, 512], BF, tag="qraw",
                                          name=f"qraw{idx}_{tc4}")
                    nc.scalar.copy(qraw[:], ps[idx][:])
                    qraws.append(qraw)
                    nc.vector.tensor_mul(out=dst[:, lts], in0=ps[idx][:],
                                         in1=cos_sb[:, ts])
                for qraw, (idx, dst) in zip(qraws, dsts):
                    rot_ps = rope_psum.tile([P, 512], F32, tag="rotp")
                    nc.tensor.matmul(rot_ps[:], perm[:], qraw[:],
                                     start=True, stop=True)
                    rt = rot_pool.tile([P, 512], F32, tag="rot")
                    nc.vector.tensor_mul(out=rt[:], in0=rot_ps[:],
                                         in1=sin_sb[:, ts])
                    nc.vector.tensor_add(out=dst[:, lts], in0=dst[:, lts],
                                         in1=rt[:])
                # transpose this chunk of V: vT [j, t] -> v_sb [t, tchunk, j]
                # via DMA xbar transpose -- keeps it off the PE/DVE/ACT
                # engines and off the rope PSUM banks entirely
                for tt in range(4 * tc4, 4 * (tc4 + 1)):
                    nc.scalar.dma_start(v_sb[cb][:, tt - 8 * cb, :],
                                        vT_tmp[:, P * tt:P * (tt + 1)],
                                        transpose=True)

        # ----- long-lived attention output (allocated after phase 1 frees)
        at_pool = stack.enter_context(tc.tile_pool(name="at", bufs=1))
        ATn = [at_pool.tile([P, T], BF, tag=f"ATn{h}", name=f"ATn{h}")
               for h in range(NHL)]
        outb_pool = stack.enter_context(tc.tile_pool(name="ob", bufs=2))

        # o_proj weights load at phase-2 start on the sync queue (idle by
        # now): the 8MB of HBM traffic lands during attention, well
        # before the first oproj group needs it
        for g in range(NHL):
            nc.sync.dma_start(wo_sb[:, g, :], woT_r[:, g, :])

        # ---------------- phase 2: causal GQA attention --------------
        if True:
            with (
                tc.tile_pool(name="pexp", bufs=8) as pexp_pool,
                tc.tile_pool(name="araw", bufs=8) as araw_pool,
                tc.tile_pool(name="dens", bufs=8) as den_small,
                tc.tile_pool(name="rden", bufs=3) as rden_pool,
                tc.tile_pool(name="stp", bufs=3, space="PSUM") as st_psum,
                tc.tile_pool(name="atp", bufs=2, space="PSUM") as at_psum,
                tc.tile_pool(name="dnp", bufs=1, space="PSUM") as den_psum,
                tc.tile_pool(name="opp", bufs=2, space="PSUM") as op_psum,
            ):
                # per (b, qt) group state: at_raw tiles + den reciprocals
                grp_state = {}

                def evac_group(b, qt, h, at_ps, den_ps):
                    """copy one finished (b, qt, h) accumulation to SBUF"""
                    st_ = grp_state[(b, qt)]
                    araw = araw_pool.tile([P, 512], F32, tag="araw",
                                          name=f"araw{b}_{qt}_{h}")
                    nc.scalar.copy(araw[:], at_ps[:])
                    den_c = den_small.tile([1, 512], F32, tag="den",
                                           name=f"den{b}_{qt}_{h}")
                    nc.scalar.copy(den_c[:], den_ps[:])
                    rec = den_small.tile([1, 512], F32, tag="rec",
                                         name=f"rec{b}_{qt}_{h}")
                    nc.vector.reciprocal_approx_fast(rec[:], den_c[:])
                    st_["araw"][h] = araw
                    st_["rec"][h] = rec

                def flush(item):
                    b, qt, h, a0, qo0, nk, px0, at_ps, den_ps = item
                    nc.tensor.matmul(
                        at_ps[:, qo0:], v_sb[b][:, a0, :],
                        px0[:, qo0:], start=(a0 == 0), stop=(a0 == nk - 1))
                    nc.tensor.matmul(
                        den_ps[:, qo0:], ones_k[:],
                        px0[:, qo0:], start=(a0 == 0), stop=(a0 == nk - 1))
                    if a0 == nk - 1:
                        evac_group(b, qt, h, at_ps, den_ps)

                pending = []       # score tiles awaiting their AV/den matmul

                def attn_group(b, qt):
                    """issue the 4 heads' score/exp/AV/den for one q tile"""
                    grp_state[(b, qt)] = {"araw": {}, "rec": {}}
                    nk = 4 * qt + 4
                    for h in range(NHL):
                        qTb = qT[h][b][:]
                        kTb = kT[b][:]
                        at_ps = at_psum.tile([P, 512], F32, tag="at")
                        den_ps = den_psum.tile([1, 512], F32, tag="den")
                        for a in range(nk):
                            qoff = max(0, P * a - 512 * qt)
                            diag = P * a >= 512 * qt
                            st = st_psum.tile([P, 512], F32, tag="st")
                            nc.tensor.matmul(
                                st[:, qoff:],
                                kTb[:, P * a:P * (a + 1)],
                                qTb[:, 512 * qt + qoff:512 * (qt + 1)],
                                start=True, stop=not diag)
                            if diag:
                                # st[k, q] += -1e9 for k > q in the
                                # diagonal 128x128 block
                                nc.tensor.matmul(
                                    st[:, qoff:qoff + P], triW[:],
                                    ident[:], start=False, stop=True)
                            pexp = pexp_pool.tile([P, 512], BF, tag="pexp")
                            nc.scalar.activation(
                                pexp[:, qoff:], st[:, qoff:], EXP,
                                scale=SM)
                            pending.append((b, qt, h, a, qoff, nk,
                                            pexp, at_ps, den_ps))
                            if len(pending) == 5:
                                flush(pending.pop(0))

                def normalize_group(b, qt):
                    """broadcast the reciprocals, scale, store to ATn"""
                    st_ = grp_state[(b, qt)]
                    cs = slice(S * b + 512 * qt, S * b + 512 * (qt + 1))
                    for h in range(NHL):
                        rden = rden_pool.tile([P, 512], F32, tag="rden")
                        nc.gpsimd.partition_broadcast(rden[:],
                                                      st_["rec"][h][:])
                        nc.vector.tensor_mul(
                            out=ATn[h][:, cs],
                            in0=st_["araw"][h][:], in1=rden[:])

                def oproj_group(b, qt):
                    """o_proj for the 4 token tiles of one (b, qt) group"""
                    for t16 in range(8 * b + 4 * qt, 8 * b + 4 * qt + 4):
                        ob = outb_pool.tile([P, HD], BF, tag="ob")
                        for ot in range(HD // 512):
                            ps = op_psum.tile([P, 512], F32, tag="op")
                            for j in range(NHL):
                                nc.tensor.matmul(
                                    ps[:], ATn[j][:, P * t16:P * (t16 + 1)],
                                    wo_sb[:, j, 512 * ot:512 * (ot + 1)],
                                    start=(j == 0), stop=(j == NHL - 1))
                            # alternate evacuation between DVE and ACT so
                            # neither FIFO's latency gates the two PSUM
                            # banks' turnover
                            if ot % 2 == 0:
                                nc.vector.tensor_copy(
                                    ob[:, 512 * ot:512 * (ot + 1)], ps[:])
                            else:
                                nc.scalar.copy(
                                    ob[:, 512 * ot:512 * (ot + 1)], ps[:])
                            if ot == 3:
                                nc.sync.dma_start(
                                    out[P * t16:P * (t16 + 1), 0:HD // 2],
                                    ob[:, 0:HD // 2])
                        nc.sync.dma_start(
                            out[P * t16:P * (t16 + 1), HD // 2:], ob[:, HD // 2:])

                groups = [(b, qt) for b in range(B) for qt in range(S // 512)]
                for gi, (b, qt) in enumerate(groups):
                    # oproj(gi-2) issues BEFORE attn(gi): its ~27µs of PE
                    # work becomes the transition filler, so batch-1
                    # attention starts that much later and never waits on
                    # the chunk-2/3 rope chain draining through the DVE
                    if gi >= 2:
                        oproj_group(*groups[gi - 2])
                    attn_group(b, qt)
                    # group gi-1's last evacuation lands during this
                    # group's first flushes, so its normalize can issue now
                    if gi >= 1:
                        normalize_group(*groups[gi - 1])
                while pending:
                    flush(pending.pop(0))
                normalize_group(*groups[-1])
                oproj_group(*groups[-2])
                oproj_group(*groups[-1])

    nc.compile()
    return nc


_NC = None


def _get_nc():
    global _NC
    if _NC is None:
        _NC = build_nc()
    return _NC


def make_in_maps(hidden_states, cos, sin, wq, wk, wv, wo):
    bf16 = ml_dtypes.bfloat16
    hidden_states = np.asarray(hidden_states, np.float32)
    cos = np.asarray(cos, np.float32)
    sin = np.asarray(sin, np.float32)
    wq = np.asarray(wq, np.float32)
    wk = np.asarray(wk, np.float32)
    wv = np.asarray(wv, np.float32)
    wo = np.asarray(wo, np.float32)

    HT = np.ascontiguousarray(hidden_states.T.astype(bf16))
    cosT = cos.T
    sinT = sin.T
    cosF = np.ascontiguousarray(np.concatenate([cosT, cosT], 0))
    sinF = np.ascontiguousarray(np.concatenate([-sinT, sinT], 0))

    in_maps = []
    for c in range(N_CORES):
        wq_c = wq[NHL * P * c:NHL * P * (c + 1)]
        wk_c = wk[P * c:P * (c + 1)]
        wv_c = wv[P * c:P * (c + 1)]
        wqkvT = np.ascontiguousarray(
            np.concatenate([wq_c, wk_c, wv_c], 0).T.astype(bf16))
        woT = np.ascontiguousarray(
            wo[:, NHL * P * c:NHL * P * (c + 1)].T.astype(bf16))
        in_maps.append(dict(hiddenT=HT, wqkvT=wqkvT, woT=woT,
                            cosF=cosF, sinF=sinF))
    return in_maps


def kernel(hidden_states, cos, sin, wq, wk, wv, wo, batch, seq_len):
    assert int(batch) == B and int(seq_len) == S
    nc = _get_nc()
    in_maps = make_in_maps(hidden_states, cos, sin, wq, wk, wv, wo)
    res = run_bass_kernel_spmd(nc, in_maps, core_ids=list(range(N_CORES)))
    acc = res.results[0]["out"].astype(np.float32)
    for c in range(1, N_CORES):
        acc += res.results[c]["out"].astype(np.float32)
    return acc


# revision 54
# speedup vs baseline: 1.0317x; 1.0038x over previous
"""Tensor-parallel FlashLlamaAttention kernel for 8 Trainium2 NeuronCores.

Sharding: each core owns 4 query heads (512 proj dims) and 1 kv head
(128 dims). Per-core device program computes qkv projection (+RoPE),
causal GQA attention and its o_proj partial product; the 8 partial
[2048, 4096] outputs are summed on the host (replaces the all-reduce).

Device-side layouts are all "feature on partitions" (transposed), so the
host wrapper pre-transposes hidden_states and the weight shards.
All matmul operands are bf16 (f32 PSUM accumulation): bf16 enables the
PE fast-weight-load path, so LDWEIGHTS hides behind the matmul stream
instead of serializing with it (f32 weights load at 1 elem/cycle and
cost ~180ns per 128x128 tile). Rotate-half for RoPE runs on the PE as a
permutation matmul (DVE cannot move data across partitions).

Softmax denominators are batched per (batch, q-tile) into a [4, 512]
tile and inverted with one reciprocal_approx_fast (the plain DVE
reciprocal costs ~3.3us per call); attention PSUM is evacuated with
plain copies so the normalization chain stays off the PE critical path.
"""
import sys

sys.path.insert(0, "/opt/trn_rl_repo")

from contextlib import ExitStack

import numpy as np
import ml_dtypes

import concourse.bass as bass
import concourse.bacc as bacc
import concourse.mybir as mybir
import concourse.tile as tile
from concourse.bass_utils import run_bass_kernel_spmd
from concourse.masks import make_identity

F32 = mybir.dt.float32
BF = mybir.dt.bfloat16
EXP = mybir.ActivationFunctionType.Exp

P = 128          # partitions / head dim
T = 2048         # total tokens (B * S)
S = 1024         # seq len per batch
B = 2
HD = 4096        # hidden dim
NHL = 4          # local query heads per core
DQKV = NHL * P + P + P  # 768 local projection dims (4q + k + v)
SM = float(P) ** -0.5

N_CORES = 8


def build_nc():
    nc = bacc.Bacc("TRN2", target_bir_lowering=False, debug=False,
                   num_devices=N_CORES)

    hiddenT = nc.dram_tensor("hiddenT", [HD, T], BF, kind="ExternalInput").ap()
    wqkvT = nc.dram_tensor("wqkvT", [HD, DQKV], BF, kind="ExternalInput").ap()
    woT = nc.dram_tensor("woT", [NHL * P, HD], BF, kind="ExternalInput").ap()
    cosF = nc.dram_tensor("cosF", [P, T], F32, kind="ExternalInput").ap()
    sinF = nc.dram_tensor("sinF", [P, T], F32, kind="ExternalInput").ap()
    out = nc.dram_tensor("out", [T, HD], BF, kind="ExternalOutput").ap()

    with tile.TileContext(nc) as tc, ExitStack() as stack:
        const = stack.enter_context(tc.tile_pool(name="const", bufs=1))
        ident_f = const.tile([P, P], F32)
        make_identity(nc, ident_f[:])
        ident = const.tile([P, P], BF)
        nc.vector.tensor_copy(ident[:], ident_f[:])
        # rotate-half permutation: perm[k, i] = 1 iff |k - i| == 64
        perm = const.tile([P, P], BF)
        nc.vector.tensor_copy(perm[:, 0:64], ident[:, 64:128])
        nc.vector.tensor_copy(perm[:, 64:128], ident[:, 0:64])
        ones_f32 = const.tile([P, 1], F32)
        nc.vector.memset(ones_f32[:], 1.0)
        ones_k = const.tile([P, 1], BF)
        nc.vector.tensor_copy(ones_k[:], ones_f32[:])
        # causal additive mask weight: triW[p, m] = -1e9 iff m > p.
        # Accumulated into the diagonal score tiles on the PE via
        # matmul(st, triW, ident) so the DVE never touches pexp.
        tri_f = const.tile([P, P], F32)
        nc.gpsimd.memset(tri_f[:], -1e9)
        nc.gpsimd.affine_select(
            out=tri_f[:], in_=tri_f[:], compare_op=mybir.AluOpType.is_ge,
            fill=0.0, base=-1, pattern=[[1, P]], channel_multiplier=-1)
        triW = const.tile([P, P], BF)
        nc.vector.tensor_copy(triW[:], tri_f[:])

        # long-lived activations, split per batch so attention on batch 0
        # never waits (tile-granular deps) on batch 1's rope/transpose work
        qkv_pool = stack.enter_context(tc.tile_pool(name="qkv", bufs=1))
        qT = [[qkv_pool.tile([P, S], BF, tag=f"qT{h}_{b}", name=f"qT{h}_{b}")
               for b in range(B)] for h in range(NHL)]
        kT = [qkv_pool.tile([P, S], BF, tag=f"kT{b}", name=f"kT{b}")
              for b in range(B)]
        v_pool = stack.enter_context(tc.tile_pool(name="v", bufs=1))
        v_sb = [v_pool.tile([P, S // P, P], BF, tag=f"v_sb{b}",
                            name=f"v_sb{b}") for b in range(B)]

        # o_proj weights tile; DMA issued inside phase 1 after the
        # wqkv/cos/sin loads so it drains during phase 1 without
        # delaying the first projection matmuls
        w2_pool = stack.enter_context(tc.tile_pool(name="w2", bufs=1))
        wo_sb = w2_pool.tile([P, NHL, HD], BF)
        woT_r = woT.rearrange("(a p) o -> p a o", p=P)

        # ---------------- phase 1: qkv projection + rope -----------------
        with (
            tc.tile_pool(name="cs", bufs=1) as cs_pool,
            tc.tile_pool(name="w1", bufs=1) as w1_pool,
            tc.tile_pool(name="xt", bufs=12) as xt_pool,
            tc.tile_pool(name="rot", bufs=2) as rot_pool,
            tc.tile_pool(name="qraw", bufs=5) as qraw_pool,
            tc.tile_pool(name="vtmp", bufs=1) as vtmp_pool,
            tc.tile_pool(name="pps", bufs=1, space="PSUM") as proj_psum,
            tc.tile_pool(name="rps", bufs=2, space="PSUM") as rope_psum,
        ):
            # one tile per 4-ho weight group: the first matmuls only
            # depend on the first group's DMA, not the whole 6 MB load
            # (deps are tile-granular)
            wqkvT_r = wqkvT.rearrange("(a p) j -> p a j", p=P)
            wqkv_g = []
            for g in range(8):
                wg = w1_pool.tile([P, 4, DQKV], BF, tag=f"wq{g}",
                                  name=f"wq{g}")
                nc.scalar.dma_start(wg[:], wqkvT_r[:, 4 * g:4 * (g + 1), :])
                wqkv_g.append(wg)
            cos_sb = cs_pool.tile([P, T], F32)
            sin_sb = cs_pool.tile([P, T], F32)
            nc.scalar.dma_start(cos_sb[:], cosF[:])
            nc.scalar.dma_start(sin_sb[:], sinF[:])
            vT_tmp = vtmp_pool.tile([P, T], BF, tag="vT", name="vT_tmp")

            for tc4 in range(T // 512):
                ts = slice(512 * tc4, 512 * (tc4 + 1))
                cb = tc4 // 2          # batch this chunk belongs to
                lts = slice(512 * (tc4 % 2), 512 * (tc4 % 2 + 1))
                dsts = [(h, qT[h][cb]) for h in range(NHL)] + [(4, kT[cb])]
                ps = [proj_psum.tile([P, 512], F32, tag=f"pps{ot}",
                                     name=f"pps{ot}_{tc4}")
                      for ot in range(6)]
                for ho in range(HD // P):
                    xt = xt_pool.tile([P, 512], BF, tag="xt")
                    nc.sync.dma_start(xt[:], hiddenT[P * ho:P * (ho + 1), ts])
                    for ot in range(6):
                        nc.tensor.matmul(
                            ps[ot][:],
                            wqkv_g[ho // 4][:, ho % 4, P * ot:P * (ot + 1)],
                            xt[:], start=(ho == 0), stop=(ho == HD // P - 1))
                # evacuate all 6 PSUM banks first (qraw copy + cos-mult
                # are each bank's only readers), so the next chunk's
                # matmuls unblock ASAP; the rot/sin/add RoPE work runs
                # afterwards off-PSUM
                nc.vector.tensor_copy(vT_tmp[:, ts], ps[5][:])
                qraws = []
                for idx, dst in dsts:
                    qraw = qraw_pool.tile([P, 512], BF, tag="qraw",
                                          name=f"qraw{idx}_{tc4}")
                    nc.scalar.copy(qraw[:], ps[idx][:])
                    qraws.append(qraw)
                    nc.vector.tensor_mul(out=dst[:, lts], in0=ps[idx][:],
                                         in1=cos_sb[:, ts])
                for qraw, (idx, dst) in zip(qraws, dsts):
                    rot_ps = rope_psum.tile([P, 512], F32, tag="rotp")
                    nc.tensor.matmul(rot_ps[:], perm[:], qraw[:],
                                     start=True, stop=True)
                    rt = rot_pool.tile([P, 512], F32, tag="rot")
                    nc.vector.tensor_mul(out=rt[:], in0=rot_ps[:],
                                         in1=sin_sb[:, ts])
                    nc.vector.tensor_add(out=dst[:, lts], in0=dst[:, lts],
                                         in1=rt[:])
                # transpose this chunk of V: vT [j, t] -> v_sb [t, tchunk, j]
                # via DMA xbar transpose -- keeps it off the PE/DVE/ACT
                # engines and off the rope PSUM banks entirely
                # batched per half (xbar-mode switches between transpose
                # and plain-copy DMAs serialize the queues, so keep them
                # off the mid-chunk xt stream): batch-0's eight issue at
                # chunk-1 end on scalar (blocks only chunk-2 qraw copies,
                # not needed for ~40us); batch-1's at chunk-3 end on
                # sync, where nothing queues behind them. wo rides sync
                # at chunk-2 end, ahead of the transposes.
                if tc4 == 1:
                    for tt in range(8):
                        nc.scalar.dma_start(v_sb[0][:, tt, :],
                                            vT_tmp[:, P * tt:P * (tt + 1)],
                                            transpose=True)
                elif tc4 == 2:
                    for g in range(NHL):
                        nc.sync.dma_start(wo_sb[:, g, :], woT_r[:, g, :])
                elif tc4 == 3:
                    for tt in range(8, 16):
                        nc.sync.dma_start(v_sb[1][:, tt - 8, :],
                                          vT_tmp[:, P * tt:P * (tt + 1)],
                                          transpose=True)

        # ----- long-lived attention output (allocated after phase 1 frees)
        at_pool = stack.enter_context(tc.tile_pool(name="at", bufs=1))
        ATn = [at_pool.tile([P, T], BF, tag=f"ATn{h}", name=f"ATn{h}")
               for h in range(NHL)]
        outb_pool = stack.enter_context(tc.tile_pool(name="ob", bufs=2))

        # ---------------- phase 2: causal GQA attention --------------
        if True:
            with (
                tc.tile_pool(name="pexp", bufs=8) as pexp_pool,
                tc.tile_pool(name="araw", bufs=8) as araw_pool,
                tc.tile_pool(name="dens", bufs=8) as den_small,
                tc.tile_pool(name="rden", bufs=3) as rden_pool,
                tc.tile_pool(name="stp", bufs=3, space="PSUM") as st_psum,
                tc.tile_pool(name="atp", bufs=2, space="PSUM") as at_psum,
                tc.tile_pool(name="dnp", bufs=1, space="PSUM") as den_psum,
                tc.tile_pool(name="opp", bufs=2, space="PSUM") as op_psum,
            ):
                # per (b, qt) group state: at_raw tiles + den reciprocals
                grp_state = {}

                def evac_group(b, qt, h, at_ps, den_ps):
                    """copy one finished (b, qt, h) accumulation to SBUF"""
                    st_ = grp_state[(b, qt)]
                    araw = araw_pool.tile([P, 512], F32, tag="araw",
                                          name=f"araw{b}_{qt}_{h}")
                    nc.scalar.copy(araw[:], at_ps[:])
                    den_c = den_small.tile([1, 512], F32, tag="den",
                                           name=f"den{b}_{qt}_{h}")
                    nc.scalar.copy(den_c[:], den_ps[:])
                    rec = den_small.tile([1, 512], F32, tag="rec",
                                         name=f"rec{b}_{qt}_{h}")
                    nc.vector.reciprocal_approx_fast(rec[:], den_c[:])
                    st_["araw"][h] = araw
                    st_["rec"][h] = rec

                def flush(item):
                    b, qt, h, a0, qo0, nk, px0, at_ps, den_ps = item
                    nc.tensor.matmul(
                        at_ps[:, qo0:], v_sb[b][:, a0, :],
                        px0[:, qo0:], start=(a0 == 0), stop=(a0 == nk - 1))
                    nc.tensor.matmul(
                        den_ps[:, qo0:], ones_k[:],
                        px0[:, qo0:], start=(a0 == 0), stop=(a0 == nk - 1))
                    if a0 == nk - 1:
                        evac_group(b, qt, h, at_ps, den_ps)

                pending = []       # score tiles awaiting their AV/den matmul

                def attn_group(b, qt):
                    """issue the 4 heads' score/exp/AV/den for one q tile"""
                    grp_state[(b, qt)] = {"araw": {}, "rec": {}}
                    nk = 4 * qt + 4
                    for h in range(NHL):
                        qTb = qT[h][b][:]
                        kTb = kT[b][:]
                        at_ps = at_psum.tile([P, 512], F32, tag="at")
                        den_ps = den_psum.tile([1, 512], F32, tag="den")
                        for a in range(nk):
                            qoff = max(0, P * a - 512 * qt)
                            diag = P * a >= 512 * qt
                            st = st_psum.tile([P, 512], F32, tag="st")
                            nc.tensor.matmul(
                                st[:, qoff:],
                                kTb[:, P * a:P * (a + 1)],
                                qTb[:, 512 * qt + qoff:512 * (qt + 1)],
                                start=True, stop=not diag)
                            if diag:
                                # st[k, q] += -1e9 for k > q in the
                                # diagonal 128x128 block
                                nc.tensor.matmul(
                                    st[:, qoff:qoff + P], triW[:],
                                    ident[:], start=False, stop=True)
                            pexp = pexp_pool.tile([P, 512], BF, tag="pexp")
                            nc.scalar.activation(
                                pexp[:, qoff:], st[:, qoff:], EXP,
                                scale=SM)
                            pending.append((b, qt, h, a, qoff, nk,
                                            pexp, at_ps, den_ps))
                            if len(pending) == 5:
                                flush(pending.pop(0))

                def normalize_group(b, qt):
                    """broadcast the reciprocals, scale, store to ATn"""
                    st_ = grp_state[(b, qt)]
                    cs = slice(S * b + 512 * qt, S * b + 512 * (qt + 1))
                    for h in range(NHL):
                        rden = rden_pool.tile([P, 512], F32, tag="rden")
                        nc.gpsimd.partition_broadcast(rden[:],
                                                      st_["rec"][h][:])
                        nc.vector.tensor_mul(
                            out=ATn[h][:, cs],
                            in0=st_["araw"][h][:], in1=rden[:])

                def oproj_group(b, qt):
                    """o_proj for the 4 token tiles of one (b, qt) group"""
                    for t16 in range(8 * b + 4 * qt, 8 * b + 4 * qt + 4):
                        ob = outb_pool.tile([P, HD], BF, tag="ob")
                        for ot in range(HD // 512):
                            ps = op_psum.tile([P, 512], F32, tag="op")
                            for j in range(NHL):
                                nc.tensor.matmul(
                                    ps[:], ATn[j][:, P * t16:P * (t16 + 1)],
                                    wo_sb[:, j, 512 * ot:512 * (ot + 1)],
                                    start=(j == 0), stop=(j == NHL - 1))
                            # alternate evacuation between DVE and ACT so
                            # neither FIFO's latency gates the two PSUM
                            # banks' turnover
                            if ot % 2 == 0:
                                nc.vector.tensor_copy(
                                    ob[:, 512 * ot:512 * (ot + 1)], ps[:])
                            else:
                                nc.scalar.copy(
                                    ob[:, 512 * ot:512 * (ot + 1)], ps[:])
                            if ot == 3:
                                nc.sync.dma_start(
                                    out[P * t16:P * (t16 + 1), 0:HD // 2],
                                    ob[:, 0:HD // 2])
                        nc.sync.dma_start(
                            out[P * t16:P * (t16 + 1), HD // 2:], ob[:, HD // 2:])

                groups = [(b, qt) for b in range(B) for qt in range(S // 512)]
                for gi, (b, qt) in enumerate(groups):
                    # oproj(gi-2) issues BEFORE attn(gi): its ~27µs of PE
                    # work becomes the transition filler, so batch-1
                    # attention starts that much later and never waits on
                    # the chunk-2/3 rope chain draining through the DVE
                    if gi >= 2:
                        oproj_group(*groups[gi - 2])
                    attn_group(b, qt)
                    # group gi-1's last evacuation lands during this
                    # group's first flushes, so its normalize can issue now
                    if gi >= 1:
                        normalize_group(*groups[gi - 1])
                while pending:
                    flush(pending.pop(0))
                normalize_group(*groups[-1])
                oproj_group(*groups[-2])
                oproj_group(*groups[-1])

    nc.compile()
    return nc


_NC = None


def _get_nc():
    global _NC
    if _NC is None:
        _NC = build_nc()
    return _NC


def make_in_maps(hidden_states, cos, sin, wq, wk, wv, wo):
    bf16 = ml_dtypes.bfloat16
    hidden_states = np.asarray(hidden_states, np.float32)
    cos = np.asarray(cos, np.float32)
    sin = np.asarray(sin, np.float32)
    wq = np.asarray(wq, np.float32)
    wk = np.asarray(wk, np.float32)
    wv = np.asarray(wv, np.float32)
    wo = np.asarray(wo, np.float32)

    HT = np.ascontiguousarray(hidden_states.T.astype(bf16))
    cosT = cos.T
    sinT = sin.T
    cosF = np.ascontiguousarray(np.concatenate([cosT, cosT], 0))
    sinF = np.ascontiguousarray(np.concatenate([-sinT, sinT], 0))

    in_maps = []
    for c in range(N_CORES):
        wq_c = wq[NHL * P * c:NHL * P * (c + 1)]
        wk_c = wk[P * c:P * (c + 1)]
        wv_c = wv[P * c:P * (c + 1)]
        wqkvT = np.ascontiguousarray(
            np.concatenate([wq_c, wk_c, wv_c], 0).T.astype(bf16))
        woT = np.ascontiguousarray(
            wo[:, NHL * P * c:NHL * P * (c + 1)].T.astype(bf16))
        in_maps.append(dict(hiddenT=HT, wqkvT=wqkvT, woT=woT,
                            cosF=cosF, sinF=sinF))
    return in_maps


def kernel(hidden_states, cos, sin, wq, wk, wv, wo, batch, seq_len):
    assert int(batch) == B and int(seq_len) == S
    nc = _get_nc()
    in_maps = make_in_maps(hidden_states, cos, sin, wq, wk, wv, wo)
    res = run_bass_kernel_spmd(nc, in_maps, core_ids=list(range(N_CORES)))
    acc = res.results[0]["out"].astype(np.float32)
    for c in range(1, N_CORES):
        acc += res.results[c]["out"].astype(np.float32)
    return acc


# revision 58
# speedup vs baseline: 1.0433x; 1.0113x over previous
"""Tensor-parallel FlashLlamaAttention kernel for 8 Trainium2 NeuronCores.

Sharding: each core owns 4 query heads (512 proj dims) and 1 kv head
(128 dims). Per-core device program computes qkv projection (+RoPE),
causal GQA attention and its o_proj partial product; the 8 partial
[2048, 4096] outputs are summed on the host (replaces the all-reduce).

Device-side layouts are all "feature on partitions" (transposed), so the
host wrapper pre-transposes hidden_states and the weight shards.
All matmul operands are bf16 (f32 PSUM accumulation): bf16 enables the
PE fast-weight-load path, so LDWEIGHTS hides behind the matmul stream
instead of serializing with it (f32 weights load at 1 elem/cycle and
cost ~180ns per 128x128 tile). Rotate-half for RoPE runs on the PE as a
permutation matmul (DVE cannot move data across partitions).

Softmax denominators are batched per (batch, q-tile) into a [4, 512]
tile and inverted with one reciprocal_approx_fast (the plain DVE
reciprocal costs ~3.3us per call); attention PSUM is evacuated with
plain copies so the normalization chain stays off the PE critical path.
"""
import sys

sys.path.insert(0, "/opt/trn_rl_repo")

from contextlib import ExitStack

import numpy as np
import ml_dtypes

import concourse.bass as bass
import concourse.bacc as bacc
import concourse.mybir as mybir
import concourse.tile as tile
from concourse.bass_utils import run_bass_kernel_spmd
from concourse.masks import make_identity

F32 = mybir.dt.float32
BF = mybir.dt.bfloat16
EXP = mybir.ActivationFunctionType.Exp

P = 128          # partitions / head dim
T = 2048         # total tokens (B * S)
S = 1024         # seq len per batch
B = 2
HD = 4096        # hidden dim
NHL = 4          # local query heads per core
DQKV = NHL * P + P + P  # 768 local projection dims (4q + k + v)
SM = float(P) ** -0.5

N_CORES = 8


def build_nc():
    nc = bacc.Bacc("TRN2", target_bir_lowering=False, debug=False,
                   num_devices=N_CORES)

    hiddenT = nc.dram_tensor("hiddenT", [HD, T], BF, kind="ExternalInput").ap()
    wqkvT = nc.dram_tensor("wqkvT", [HD, DQKV], BF, kind="ExternalInput").ap()
    woT = nc.dram_tensor("woT", [NHL * P, HD], BF, kind="ExternalInput").ap()
    cosF = nc.dram_tensor("cosF", [P, T], F32, kind="ExternalInput").ap()
    sinF = nc.dram_tensor("sinF", [P, T], F32, kind="ExternalInput").ap()
    out = nc.dram_tensor("out", [T, HD], BF, kind="ExternalOutput").ap()

    with tile.TileContext(nc) as tc, ExitStack() as stack:
        const = stack.enter_context(tc.tile_pool(name="const", bufs=1))
        ident_f = const.tile([P, P], F32)
        make_identity(nc, ident_f[:])
        ident = const.tile([P, P], BF)
        nc.vector.tensor_copy(ident[:], ident_f[:])
        # rotate-half permutation: perm[k, i] = 1 iff |k - i| == 64
        perm = const.tile([P, P], BF)
        nc.vector.tensor_copy(perm[:, 0:64], ident[:, 64:128])
        nc.vector.tensor_copy(perm[:, 64:128], ident[:, 0:64])
        ones_f32 = const.tile([P, 1], F32)
        nc.vector.memset(ones_f32[:], 1.0)
        ones_k = const.tile([P, 1], BF)
        nc.vector.tensor_copy(ones_k[:], ones_f32[:])
        # causal additive mask weight: triW[p, m] = -1e9 iff m > p.
        # Accumulated into the diagonal score tiles on the PE via
        # matmul(st, triW, ident) so the DVE never touches pexp.
        tri_f = const.tile([P, P], F32)
        nc.gpsimd.memset(tri_f[:], -1e9)
        nc.gpsimd.affine_select(
            out=tri_f[:], in_=tri_f[:], compare_op=mybir.AluOpType.is_ge,
            fill=0.0, base=-1, pattern=[[1, P]], channel_multiplier=-1)
        triW = const.tile([P, P], BF)
        nc.vector.tensor_copy(triW[:], tri_f[:])

        # long-lived activations, split per batch so attention on batch 0
        # never waits (tile-granular deps) on batch 1's rope/transpose work
        qkv_pool = stack.enter_context(tc.tile_pool(name="qkv", bufs=1))
        qT = [[qkv_pool.tile([P, S], BF, tag=f"qT{h}_{b}", name=f"qT{h}_{b}")
               for b in range(B)] for h in range(NHL)]
        kT = [qkv_pool.tile([P, S], BF, tag=f"kT{b}", name=f"kT{b}")
              for b in range(B)]
        v_pool = stack.enter_context(tc.tile_pool(name="v", bufs=1))
        v_sb = [v_pool.tile([P, S // P, P], BF, tag=f"v_sb{b}",
                            name=f"v_sb{b}") for b in range(B)]

        # o_proj weights tile; DMA issued inside phase 1 after the
        # wqkv/cos/sin loads so it drains during phase 1 without
        # delaying the first projection matmuls
        w2_pool = stack.enter_context(tc.tile_pool(name="w2", bufs=1))
        wo_sb = w2_pool.tile([P, NHL, HD], BF)
        woT_r = woT.rearrange("(a p) o -> p a o", p=P)

        # ---------------- phase 1: qkv projection + rope -----------------
        with (
            tc.tile_pool(name="cs", bufs=1) as cs_pool,
            tc.tile_pool(name="w1", bufs=1) as w1_pool,
            tc.tile_pool(name="xt", bufs=16) as xt_pool,
            tc.tile_pool(name="rot", bufs=2) as rot_pool,
            tc.tile_pool(name="qraw", bufs=5) as qraw_pool,
            tc.tile_pool(name="vtmp", bufs=1) as vtmp_pool,
            tc.tile_pool(name="pps", bufs=1, space="PSUM") as proj_psum,
            tc.tile_pool(name="rps", bufs=2, space="PSUM") as rope_psum,
        ):
            # one tile per 4-ho weight group: the first matmuls only
            # depend on the first group's DMA, not the whole 6 MB load
            # (deps are tile-granular)
            wqkvT_r = wqkvT.rearrange("(a p) j -> p a j", p=P)
            # group 0 split per-ho so the opening matmul waits on a 196KB
            # DMA instead of 786KB; the rest stay 4-ho coarse
            wqkv_h = []
            for h in range(4):
                wh = w1_pool.tile([P, 1, DQKV], BF, tag=f"wh{h}",
                                  name=f"wh{h}")
                nc.scalar.dma_start(wh[:], wqkvT_r[:, h:h + 1, :])
                wqkv_h.append(wh)
            wqkv_g = []
            for g in range(1, 8):
                wg = w1_pool.tile([P, 4, DQKV], BF, tag=f"wq{g}",
                                  name=f"wq{g}")
                nc.scalar.dma_start(wg[:], wqkvT_r[:, 4 * g:4 * (g + 1), :])
                wqkv_g.append(wg)
            cos_sb = cs_pool.tile([P, T], F32)
            sin_sb = cs_pool.tile([P, T], F32)
            nc.scalar.dma_start(cos_sb[:], cosF[:])
            nc.scalar.dma_start(sin_sb[:], sinF[:])
            vT_tmp = vtmp_pool.tile([P, T], BF, tag="vT", name="vT_tmp")

            for tc4 in range(T // 512):
                ts = slice(512 * tc4, 512 * (tc4 + 1))
                cb = tc4 // 2          # batch this chunk belongs to
                lts = slice(512 * (tc4 % 2), 512 * (tc4 % 2 + 1))
                dsts = [(h, qT[h][cb]) for h in range(NHL)] + [(4, kT[cb])]
                ps = [proj_psum.tile([P, 512], F32, tag=f"pps{ot}",
                                     name=f"pps{ot}_{tc4}")
                      for ot in range(6)]
                for ho in range(HD // P):
                    xt = xt_pool.tile([P, 512], BF, tag="xt")
                    nc.sync.dma_start(xt[:], hiddenT[P * ho:P * (ho + 1), ts])
                    if ho < 4:
                        wsrc = wqkv_h[ho][:, 0, :]
                    else:
                        wsrc = wqkv_g[ho // 4 - 1][:, ho % 4, :]
                    for ot in range(6):
                        nc.tensor.matmul(
                            ps[ot][:], wsrc[:, P * ot:P * (ot + 1)],
                            xt[:], start=(ho == 0), stop=(ho == HD // P - 1))
                # evacuate all 6 PSUM banks first (qraw copy + cos-mult
                # are each bank's only readers), so the next chunk's
                # matmuls unblock ASAP; the rot/sin/add RoPE work runs
                # afterwards off-PSUM
                nc.vector.tensor_copy(vT_tmp[:, ts], ps[5][:])
                qraws = []
                for idx, dst in dsts:
                    qraw = qraw_pool.tile([P, 512], BF, tag="qraw",
                                          name=f"qraw{idx}_{tc4}")
                    nc.scalar.copy(qraw[:], ps[idx][:])
                    qraws.append(qraw)
                    nc.vector.tensor_mul(out=dst[:, lts], in0=ps[idx][:],
                                         in1=cos_sb[:, ts])
                for qraw, (idx, dst) in zip(qraws, dsts):
                    rot_ps = rope_psum.tile([P, 512], F32, tag="rotp")
                    nc.tensor.matmul(rot_ps[:], perm[:], qraw[:],
                                     start=True, stop=True)
                    rt = rot_pool.tile([P, 512], F32, tag="rot")
                    nc.vector.tensor_mul(out=rt[:], in0=rot_ps[:],
                                         in1=sin_sb[:, ts])
                    nc.vector.tensor_add(out=dst[:, lts], in0=dst[:, lts],
                                         in1=rt[:])
                # transpose this chunk of V: vT [j, t] -> v_sb [t, tchunk, j]
                # via DMA xbar transpose -- keeps it off the PE/DVE/ACT
                # engines and off the rope PSUM banks entirely
                # batched per half (xbar-mode switches between transpose
                # and plain-copy DMAs serialize the queues, so keep them
                # off the mid-chunk xt stream): batch-0's eight issue at
                # chunk-1 end on scalar (blocks only chunk-2 qraw copies,
                # not needed for ~40us); batch-1's at chunk-3 end on
                # sync, where nothing queues behind them. wo rides sync
                # at chunk-2 end, ahead of the transposes.
                if tc4 == 1:
                    for tt in range(8):
                        nc.scalar.dma_start(v_sb[0][:, tt, :],
                                            vT_tmp[:, P * tt:P * (tt + 1)],
                                            transpose=True)
                elif tc4 == 2:
                    for g in range(NHL):
                        nc.sync.dma_start(wo_sb[:, g, :], woT_r[:, g, :])
                elif tc4 == 3:
                    for tt in range(8, 16):
                        nc.sync.dma_start(v_sb[1][:, tt - 8, :],
                                          vT_tmp[:, P * tt:P * (tt + 1)],
                                          transpose=True)

        # ----- long-lived attention output (allocated after phase 1 frees)
        at_pool = stack.enter_context(tc.tile_pool(name="at", bufs=1))
        ATn = [at_pool.tile([P, T], BF, tag=f"ATn{h}", name=f"ATn{h}")
               for h in range(NHL)]
        outb_pool = stack.enter_context(tc.tile_pool(name="ob", bufs=2))

        # ---------------- phase 2: causal GQA attention --------------
        if True:
            with (
                tc.tile_pool(name="pexp", bufs=8) as pexp_pool,
                tc.tile_pool(name="araw", bufs=8) as araw_pool,
                tc.tile_pool(name="dens", bufs=8) as den_small,
                tc.tile_pool(name="rden", bufs=3) as rden_pool,
                tc.tile_pool(name="stp", bufs=3, space="PSUM") as st_psum,
                tc.tile_pool(name="atp", bufs=2, space="PSUM") as at_psum,
                tc.tile_pool(name="dnp", bufs=1, space="PSUM") as den_psum,
                tc.tile_pool(name="opp", bufs=2, space="PSUM") as op_psum,
            ):
                # per (b, qt) group state: at_raw tiles + den reciprocals
                grp_state = {}

                def evac_group(b, qt, h, at_ps, den_ps):
                    """copy one finished (b, qt, h) accumulation to SBUF"""
                    st_ = grp_state[(b, qt)]
                    araw = araw_pool.tile([P, 512], F32, tag="araw",
                                          name=f"araw{b}_{qt}_{h}")
                    nc.scalar.copy(araw[:], at_ps[:])
                    den_c = den_small.tile([1, 512], F32, tag="den",
                                           name=f"den{b}_{qt}_{h}")
                    nc.scalar.copy(den_c[:], den_ps[:])
                    rec = den_small.tile([1, 512], F32, tag="rec",
                                         name=f"rec{b}_{qt}_{h}")
                    nc.vector.reciprocal_approx_fast(rec[:], den_c[:])
                    st_["araw"][h] = araw
                    st_["rec"][h] = rec

                def flush(item):
                    b, qt, h, a0, qo0, nk, px0, at_ps, den_ps = item
                    nc.tensor.matmul(
                        at_ps[:, qo0:], v_sb[b][:, a0, :],
                        px0[:, qo0:], start=(a0 == 0), stop=(a0 == nk - 1))
                    nc.tensor.matmul(
                        den_ps[:, qo0:], ones_k[:],
                        px0[:, qo0:], start=(a0 == 0), stop=(a0 == nk - 1))
                    if a0 == nk - 1:
                        evac_group(b, qt, h, at_ps, den_ps)

                pending = []       # score tiles awaiting their AV/den matmul

                def attn_group(b, qt):
                    """issue the 4 heads' score/exp/AV/den for one q tile"""
                    grp_state[(b, qt)] = {"araw": {}, "rec": {}}
                    nk = 4 * qt + 4
                    for h in range(NHL):
                        qTb = qT[h][b][:]
                        kTb = kT[b][:]
                        at_ps = at_psum.tile([P, 512], F32, tag="at")
                        den_ps = den_psum.tile([1, 512], F32, tag="den")
                        for a in range(nk):
                            qoff = max(0, P * a - 512 * qt)
                            diag = P * a >= 512 * qt
                            st = st_psum.tile([P, 512], F32, tag="st")
                            nc.tensor.matmul(
                                st[:, qoff:],
                                kTb[:, P * a:P * (a + 1)],
                                qTb[:, 512 * qt + qoff:512 * (qt + 1)],
                                start=True, stop=not diag)
                            if diag:
                                # st[k, q] += -1e9 for k > q in the
                                # diagonal 128x128 block
                                nc.tensor.matmul(
                                    st[:, qoff:qoff + P], triW[:],
                                    ident[:], start=False, stop=True)
                            pexp = pexp_pool.tile([P, 512], BF, tag="pexp")
                            nc.scalar.activation(
                                pexp[:, qoff:], st[:, qoff:], EXP,
                                scale=SM)
                            pending.append((b, qt, h, a, qoff, nk,
                                            pexp, at_ps, den_ps))
                            if len(pending) == 5:
                                flush(pending.pop(0))

                def normalize_group(b, qt):
                    """broadcast the reciprocals, scale, store to ATn"""
                    st_ = grp_state[(b, qt)]
                    cs = slice(S * b + 512 * qt, S * b + 512 * (qt + 1))
                    for h in range(NHL):
                        rden = rden_pool.tile([P, 512], F32, tag="rden")
                        nc.gpsimd.partition_broadcast(rden[:],
                                                      st_["rec"][h][:])
                        nc.vector.tensor_mul(
                            out=ATn[h][:, cs],
                            in0=st_["araw"][h][:], in1=rden[:])

                def oproj_group(b, qt):
                    """o_proj for the 4 token tiles of one (b, qt) group"""
                    for t16 in range(8 * b + 4 * qt, 8 * b + 4 * qt + 4):
                        ob = outb_pool.tile([P, HD], BF, tag="ob")
                        for ot in range(HD // 512):
                            ps = op_psum.tile([P, 512], F32, tag="op")
                            for j in range(NHL):
                                nc.tensor.matmul(
                                    ps[:], ATn[j][:, P * t16:P * (t16 + 1)],
                                    wo_sb[:, j, 512 * ot:512 * (ot + 1)],
                                    start=(j == 0), stop=(j == NHL - 1))
                            # alternate evacuation between DVE and ACT so
                            # neither FIFO's latency gates the two PSUM
                            # banks' turnover
                            if ot % 2 == 0:
                                nc.vector.tensor_copy(
                                    ob[:, 512 * ot:512 * (ot + 1)], ps[:])
                            else:
                                nc.scalar.copy(
                                    ob[:, 512 * ot:512 * (ot + 1)], ps[:])
                            if ot in (1, 3, 5):
                                q0, q1 = 512 * (ot - 1), 512 * (ot + 1)
                                nc.sync.dma_start(
                                    out[P * t16:P * (t16 + 1), q0:q1],
                                    ob[:, q0:q1])
                        nc.sync.dma_start(
                            out[P * t16:P * (t16 + 1), 3072:], ob[:, 3072:])

                groups = [(b, qt) for b in range(B) for qt in range(S // 512)]
                for gi, (b, qt) in enumerate(groups):
                    # oproj(gi-2) issues BEFORE attn(gi): its ~27µs of PE
                    # work becomes the transition filler, so batch-1
                    # attention starts that much later and never waits on
                    # the chunk-2/3 rope chain draining through the DVE
                    if gi >= 2:
                        oproj_group(*groups[gi - 2])
                    attn_group(b, qt)
                    # group gi-1's last evacuation lands during this
                    # group's first flushes, so its normalize can issue now
                    if gi >= 1:
                        normalize_group(*groups[gi - 1])
                while pending:
                    flush(pending.pop(0))
                normalize_group(*groups[-1])
                oproj_group(*groups[-2])
                oproj_group(*groups[-1])

    nc.compile()
    return nc


_NC = None


def _get_nc():
    global _NC
    if _NC is None:
        _NC = build_nc()
    return _NC


def make_in_maps(hidden_states, cos, sin, wq, wk, wv, wo):
    bf16 = ml_dtypes.bfloat16
    hidden_states = np.asarray(hidden_states, np.float32)
    cos = np.asarray(cos, np.float32)
    sin = np.asarray(sin, np.float32)
    wq = np.asarray(wq, np.float32)
    wk = np.asarray(wk, np.float32)
    wv = np.asarray(wv, np.float32)
    wo = np.asarray(wo, np.float32)

    HT = np.ascontiguousarray(hidden_states.T.astype(bf16))
    cosT = cos.T
    sinT = sin.T
    cosF = np.ascontiguousarray(np.concatenate([cosT, cosT], 0))
    sinF = np.ascontiguousarray(np.concatenate([-sinT, sinT], 0))

    in_maps = []
    for c in range(N_CORES):
        wq_c = wq[NHL * P * c:NHL * P * (c + 1)]
        wk_c = wk[P * c:P * (c + 1)]
        wv_c = wv[P * c:P * (c + 1)]
        wqkvT = np.ascontiguousarray(
            np.concatenate([wq_c, wk_c, wv_c], 0).T.astype(bf16))
        woT = np.ascontiguousarray(
            wo[:, NHL * P * c:NHL * P * (c + 1)].T.astype(bf16))
        in_maps.append(dict(hiddenT=HT, wqkvT=wqkvT, woT=woT,
                            cosF=cosF, sinF=sinF))
    return in_maps


def kernel(hidden_states, cos, sin, wq, wk, wv, wo, batch, seq_len):
    assert int(batch) == B and int(seq_len) == S
    nc = _get_nc()
    in_maps = make_in_maps(hidden_states, cos, sin, wq, wk, wv, wo)
    res = run_bass_kernel_spmd(nc, in_maps, core_ids=list(range(N_CORES)))
    acc = res.results[0]["out"].astype(np.float32)
    for c in range(1, N_CORES):
        acc += res.results[c]["out"].astype(np.float32)
    return acc
